# revision 6
# baseline (speedup 1.0000x reference)
"""Trainium2 Bass kernel for nn_DGM_77318001263213 (dense_transformer).

Reference computation (per batch b of 16):
  dir_map = conv3x3_SAME(x, dc_w) + dc_b            [12, 64, 64]
  q = conv2x2_s2(dir_map, q_w) + q_b  -> [48, 1024]
  k = conv2x2_s2(dir_map, k_w) + k_b  -> [48, 1024]
  v = conv2x2_s2(x, v_w) + v_b        -> [1024, 1024]
  attn = softmax(q^T k, axis=-1)                    [1024, 1024]
  out[c, m] = sum_n v[c, n] * attn[m, n]            [1024, 1024]

Device mapping (data-parallel, 2 batches per core on 8 cores):
  * q,k computed as ONE composite 4x4 stride-2 convolution of x (the 3x3
    dc conv and 2x2 proj convs fold on the host) with 96 output channels.
    The conv consumes x as 4 stride-2 parity planes per half (stride-1
    innermost free dim); each plane is used ONCE (both m-half psums
    accumulate in parallel on 2 banks) so plane DMAs pace the pipe only
    at their arrival rate.
  * startup: wqk weight-group and x-plane DMAs are interleaved in exact
    consumption order, staggered across the two HWDGE queues (SP + ACT),
    so the first matmul starts ~11us in instead of ~23us.
  * the q/k path stays fp32(r) end-to-end: score errors pass through
    exp() and get amplified ~10x, bf16 there alone costs ~1e-2 rel err.
  * the v path, attention weights E, V^T tiles and the output are bf16
    (sim: 5.1e-3 global rel err vs 2e-2 gate): halves wv/output DMA and
    SBUF, and enables fast-weight-load on the PE stationary operands.
  * fp8 DoubleRow was simulated and REJECTED: e4m3 on the v conv alone
    is 4e-2 global rel err, fp8 attn 2.4e-2 -- both over the gate.
  * scores computed transposed: T[n, m] = S[m, n], E = exp(T) (|S|<=~25
    so f32 exp is safe), U^T[m, c] = sum_n E[n, m] V^T[n, c], row sums
    D[m] via ones-matmul on the bf16-accumulated esum, out^T = U^T/D as
    per-partition scale on eviction, stored bf16, host transposes+casts.
"""
import os
import sys
import types
import numpy as np
from contextlib import ExitStack

for _p in ("/opt/trn_rl_repo", "/root/.axon_site/_ro/trn_rl_repo"):
    if os.path.isdir(_p) and _p not in sys.path:
        sys.path.insert(0, _p)

import ml_dtypes
import concourse.bacc as bacc
import concourse.bass as bass
import concourse.tile as tile
import concourse.mybir as mybir
from concourse import bass_utils

F32 = mybir.dt.float32
F32R = mybir.dt.float32r
BF16 = mybir.dt.bfloat16
ts = bass.ts

NCORES = 8
BPC = 2          # batches per core
C_IN = 256
NPOS = 1024      # 32*32 output positions


def _install_ntff_hook_shim():
    """Register the axon NTFF profile hook if the image's antenv lacks it."""
    if "antenv.axon_hooks" in sys.modules:
        return
    try:
        from trn_agent_boot.trn_boot import _ntff_profile_via_ctypes
        hook = _ntff_profile_via_ctypes("/opt/axon/libaxon_pjrt.so")
    except Exception:
        hook = None
    m = types.ModuleType("antenv.axon_hooks")
    m.get_axon_ntff_profile_hook = lambda: hook
    m.set_axon_ntff_profile_hook = lambda h: None
    sys.modules["antenv.axon_hooks"] = m


def build_program():
    """Build the per-core Bacc program (same program on all 8 cores)."""
    nc = bacc.Bacc(trn_type="TRN2", target_bir_lowering=False, debug=False)

    # padded x as 4 stride-2 parity planes: xq[b, c, a*2+p, r, s] =
    # x_pad[b, c, 2r+a, 2s+p]
    xq = nc.dram_tensor("xq", [BPC, C_IN, 4, 33, 33], F32, kind="ExternalInput")
    wqk = nc.dram_tensor("wqk", [128, 32, 96], F32, kind="ExternalInput")
    wv = nc.dram_tensor("wv", [128, 8, 1024], BF16, kind="ExternalInput")
    bqk = nc.dram_tensor("bqk", [96, 1], F32, kind="ExternalInput")
    bvr = nc.dram_tensor("bvr", [128, 1024], F32, kind="ExternalInput")
    o = nc.dram_tensor("o", [BPC, 1024, 1024], BF16, kind="ExternalOutput")

    EXP = mybir.ActivationFunctionType.Exp
    COPY = mybir.ActivationFunctionType.Copy

    with tile.TileContext(nc) as tc, ExitStack() as ctx:
        const = ctx.enter_context(tc.tile_pool(name="const", bufs=1))
        xpool = ctx.enter_context(tc.tile_pool(name="xpool", bufs=16))
        xspool = ctx.enter_context(tc.tile_pool(name="xspool", bufs=16))
        qkp = ctx.enter_context(tc.tile_pool(name="qkp", bufs=1))
        epool = ctx.enter_context(tc.tile_pool(name="epool", bufs=1))
        vtpool = ctx.enter_context(tc.tile_pool(name="vtpool", bufs=1))
        outp = ctx.enter_context(tc.tile_pool(name="outp", bufs=4))
        misc = ctx.enter_context(tc.tile_pool(name="misc", bufs=2))
        ppq = ctx.enter_context(tc.tile_pool(name="ppq", bufs=1, space="PSUM"))
        ppt = ctx.enter_context(tc.tile_pool(name="ppt", bufs=1, space="PSUM"))
        ppv = ctx.enter_context(tc.tile_pool(name="ppv", bufs=2, space="PSUM"))
        ppu = ctx.enter_context(tc.tile_pool(name="ppu", bufs=2, space="PSUM"))

        # ---- persistent constants (tiles now; DMAs interleaved below) ----
        wqk_g = [
            const.tile([128, 4, 96], F32R, tag=f"wqk_g{g}", name=f"wqk_g{g}")
            for g in range(8)
        ]
        wv_t = [
            const.tile([128, 1024], BF16, tag=f"wv_sb{ck}", name=f"wv_sb{ck}")
            for ck in range(8)
        ]
        bqk_sb = const.tile([96, 1], F32, tag="bqk_sb")
        bvr_sb = const.tile([128, 1024], F32, tag="bvr_sb")
        # N=2 f32r ones for the D-sum matmuls (memset can't write f32r)
        ones_f32 = const.tile([128, 2], F32, tag="ones_f32")
        nc.vector.memset(ones_f32[:], 1.0)
        ones2 = const.tile([128, 2], F32R, tag="ones2")
        nc.scalar.copy(ones2[:], ones_f32[:])

        for b in range(BPC):
            # ---- x parity planes + (b0) wqk groups, interleaved in the
            # exact qk-conv consumption order and staggered across the two
            # HWDGE queues so planes land every ~1.7us ----
            xh = [[None] * 4 for _ in range(2)]
            for h in range(2):
                for pl in range(4):
                    g = h * 4 + pl
                    eng = nc.sync if g % 2 == 0 else nc.scalar
                    if b == 0:
                        eng.dma_start(
                            wqk_g[g][:], wqk.ap().bitcast(F32R)[:, 4 * g : 4 * g + 4, :]
                        )
                    xt = xpool.tile([128, 33, 33], F32R, tag="xh")
                    eng.dma_start(xt[:], xq.ap().bitcast(F32R)[b, ts(h, 128), pl])
                    xh[h][pl] = xt
            if b == 0:
                nc.sync.dma_start(bqk_sb[:], bqk.ap())
                for ck in range(8):
                    nc.scalar.dma_start(wv_t[ck][:], wv.ap()[:, ck, :])
                nc.scalar.dma_start(bvr_sb[:], bvr.ap())

            # space-to-depth x for the v conv, derived ON DEVICE from the
            # parity planes, cast to bf16 (fast-weight-load stationary).
            # Ordered by plane arrival (p00, p01, p02, p03, p10, ...).
            xs_c = [None] * 8
            for ck in (6, 4, 2, 0, 7, 5, 3, 1):
                t, h = divmod(ck, 2)
                dy, dx = divmod(t, 2)
                a, u2 = (dy + 1) % 2, (dy + 1) // 2
                p2, v2 = (dx + 1) % 2, (dx + 1) // 2
                xst = xspool.tile([128, 1024], BF16, tag="xs")
                srcv = xh[h][a * 2 + p2][:, u2 : u2 + 32, v2 : v2 + 32]
                dstv = xst[:].rearrange("p (a b) -> p a b", a=32)
                nc.vector.tensor_copy(dstv, srcv)
                xs_c[ck] = xst

            # ---- composite q|k conv: both m-half psums accumulate in
            # parallel so each plane is consumed once, right as it lands ----
            QK = qkp.tile([96, 1024], F32R, tag="QK")
            pq_t = [
                ppq.tile([96, 512], F32, tag=f"pq{jm}", name=f"pq{jm}")
                for jm in range(2)
            ]
            for h in range(2):
                for pl in range(4):
                    for u in range(2):
                        for v in range(2):
                            ck2 = h * 16 + pl * 4 + u * 2 + v
                            first = ck2 == 0
                            last = ck2 == 31
                            for jm in range(2):
                                rhs = xh[h][pl][
                                    :, u + 16 * jm : u + 16 * jm + 16, v : v + 32
                                ]
                                nc.tensor.matmul(
                                    pq_t[jm][:], wqk_g[ck2 // 4][:, ck2 % 4, :], rhs,
                                    start=first, stop=last,
                                )
            for jm in range(2):
                nc.vector.tensor_scalar_add(
                    QK[:, ts(jm, 512)], pq_t[jm][:], bqk_sb[:, :1]
                )
            # q rows to partition base 0 for use as the scores rhs (engine
            # accesses must start at partition 0/32/64; DMA is unrestricted)
            Qs = qkp.tile([48, 1024], F32R, tag="Qs")
            nc.sync.dma_start(Qs[:], QK[48:96, :])

            # ---- v conv (V^T tiles, bf16) interleaved with the transposed
            # scores matmuls + exps + esum partials ----
            e_sb = epool.tile([128, 8, 1024], BF16, tag="e_sb")
            vt_sb = vtpool.tile([128, 8, 1024], BF16, tag="vt_sb")
            esum = epool.tile([128, 1024], F32R, tag="esum")
            for g in range(16):
                jn, l = divmod(g, 2)
                pv_t = ppv.tile([128, 512], F32, tag="pv")
                for ck in range(8):
                    nc.tensor.matmul(
                        pv_t[:], xs_c[ck][:, ts(jn, 128)], wv_t[ck][:, ts(l, 512)],
                        start=(ck == 0), stop=(ck == 7),
                    )
                nc.vector.tensor_add(
                    vt_sb[:, jn, ts(l, 512)], pv_t[:], bvr_sb[:, ts(l, 512)]
                )
                # scores chunk g: T[n, m] = S[m, n] for n-chunk g//2, m-half
                # g%2 (k/q evicted to separate base-0 tiles -- matmul
                # operands must share base partition 0/32/64)
                sn, sm = divmod(g, 2)
                pt_t = ppt.tile([128, 512], F32, tag="pt")
                nc.tensor.matmul(
                    pt_t[:], QK[0:48, ts(sn, 128)], Qs[:, ts(sm, 512)],
                    start=True, stop=True,
                )
                nc.scalar.activation(e_sb[:, sn, ts(sm, 512)], pt_t[:], EXP)
                if g % 2 == 1:
                    if sn == 1:
                        nc.any.tensor_add(esum[:], e_sb[:, 0, :], e_sb[:, 1, :])
                    elif sn > 1:
                        nc.any.tensor_add(esum[:], esum[:], e_sb[:, sn, :])

            # ---- U^T[m, c] = sum_n E[n, m] V^T[n, c]; D[m]; out^T = U^T/D ----
            for mm in range(8):
                pd_t = ppt.tile([128, 2], F32, tag="pd")
                nc.tensor.matmul(
                    pd_t[:], esum[:, ts(mm, 128)], ones2[:], start=True, stop=True
                )
                rc = misc.tile([128, 1], F32, tag="rc")
                nc.vector.reciprocal(rc[:], pd_t[:, 0:1])
                for l in range(2):
                    pu_t = ppu.tile([128, 512], F32, tag="pu")
                    for jn in range(8):
                        nc.tensor.matmul(
                            pu_t[:], e_sb[:, jn, ts(mm, 128)], vt_sb[:, jn, ts(l, 512)],
                            start=(jn == 0), stop=(jn == 7),
                        )
                    ot = outp.tile([128, 512], BF16, tag="ot")
                    nc.scalar.activation(ot[:], pu_t[:], COPY, scale=rc[:])
                    nc.gpsimd.dma_start(o.ap()[b, ts(mm, 128), ts(l, 512)], ot[:])

    nc.compile()
    return nc


def host_weights(dc_w, dc_b, q_w, k_w, q_b, k_b, v_w, v_b):
    """Fold dc conv into q/k projections -> composite 4x4 stride-2 weights."""
    dc_w = np.asarray(dc_w, np.float32)
    dc_b = np.asarray(dc_b, np.float32)
    q_w = np.asarray(q_w, np.float32)
    k_w = np.asarray(k_w, np.float32)
    q_b = np.asarray(q_b, np.float32)
    k_b = np.asarray(k_b, np.float32)
    v_w = np.asarray(v_w, np.float32)
    v_b = np.asarray(v_b, np.float32)

    C = dc_w.shape[1]
    Wq = np.zeros((48, C, 4, 4), np.float64)
    Wk = np.zeros((48, C, 4, 4), np.float64)
    for p in range(2):
        for qq in range(2):
            qw_pq = q_w[:, :, p, qq].astype(np.float64)
            kw_pq = k_w[:, :, p, qq].astype(np.float64)
            for dy in range(3):
                for dx in range(3):
                    dcw_dd = dc_w[:, :, dy, dx].astype(np.float64)
                    Wq[:, :, p + dy, qq + dx] += qw_pq @ dcw_dd
                    Wk[:, :, p + dy, qq + dx] += kw_pq @ dcw_dd
    bq_eff = q_b + q_w.sum(axis=(2, 3)) @ dc_b
    bk_eff = k_b + k_w.sum(axis=(2, 3)) @ dc_b
    # lhsT row index = (A*4+B)*C + c', columns: k 0:48 | q 48:96
    wqk_ab = (
        np.concatenate(
            [
                Wk.transpose(2, 3, 1, 0).reshape(16 * C, 48),
                Wq.transpose(2, 3, 1, 0).reshape(16 * C, 48),
            ],
            axis=1,
        )
        .astype(np.float32)
        .reshape(32, 128, 96)  # chunk_old = (A*4+B)*2 + h
    )
    # permute chunks into device consumption order (h, pl, u, v)
    perm = []
    for h in range(2):
        for pl in range(4):
            a, p = divmod(pl, 2)
            for u in range(2):
                for v in range(2):
                    A, Bo = 2 * u + a, 2 * v + p
                    perm.append((A * 4 + Bo) * 2 + h)
    wqk = wqk_ab[perm].transpose(1, 0, 2)  # [part 128, chunk2 32, 96]
    bqk = np.concatenate([bk_eff, bq_eff]).reshape(96, 1).astype(np.float32)
    # v rhs: row = (dy*2+dx)*C + c', col = oc -- sent bf16
    wv = np.ascontiguousarray(
        v_w.transpose(2, 3, 1, 0).reshape(8, 128, 4 * C).transpose(1, 0, 2)
    ).astype(ml_dtypes.bfloat16)  # [part 128, chunk 8, oc]
    bvr = np.ascontiguousarray(np.broadcast_to(v_b, (128, 4 * C))).astype(np.float32)
    return wqk, bqk, wv, bvr


_PROGRAM = None
LAST_RESULTS = None


def _get_program():
    global _PROGRAM
    if _PROGRAM is None:
        _PROGRAM = build_program()
    return _PROGRAM


def kernel(x, dc_w, dc_b, q_w, q_b, k_w, k_b, v_w, v_b):
    _install_ntff_hook_shim()
    x = np.asarray(x, np.float32)
    B = x.shape[0]
    xp = np.pad(x, ((0, 0), (0, 0), (1, 1), (1, 1)))
    # parity planes: xq[b, c, a*2+p, r, s] = x_pad[b, c, 2r+a, 2s+p]
    xq = (
        xp.reshape(B, C_IN, 33, 2, 33, 2)
        .transpose(0, 1, 3, 5, 2, 4)
        .reshape(B, C_IN, 4, 33, 33)
    )
    wqk, bqk, wv, bvr = host_weights(dc_w, dc_b, q_w, k_w, q_b, k_b, v_w, v_b)

    nc = _get_program()
    in_maps = []
    for c in range(NCORES):
        in_maps.append(
            {
                "xq": np.ascontiguousarray(xq[BPC * c : BPC * (c + 1)]),
                "wqk": wqk,
                "wv": wv,
                "bqk": bqk,
                "bvr": bvr,
            }
        )
    res = bass_utils.run_bass_kernel_spmd(nc, in_maps, core_ids=list(range(NCORES)))
    global LAST_RESULTS
    LAST_RESULTS = res

    out = np.empty((B, 1024, 1024), np.float32)
    for c in range(NCORES):
        out[BPC * c : BPC * (c + 1)] = (
            np.asarray(res.results[c]["o"]).astype(np.float32).transpose(0, 2, 1)
        )
    return out


# revision 7
# speedup vs baseline: 1.1315x; 1.1315x over previous
"""Trainium2 Bass kernel for nn_DGM_77318001263213 (dense_transformer).

Reference computation (per batch b of 16):
  dir_map = conv3x3_SAME(x, dc_w) + dc_b            [12, 64, 64]
  q = conv2x2_s2(dir_map, q_w) + q_b  -> [48, 1024]
  k = conv2x2_s2(dir_map, k_w) + k_b  -> [48, 1024]
  v = conv2x2_s2(x, v_w) + v_b        -> [1024, 1024]
  attn = softmax(q^T k, axis=-1)                    [1024, 1024]
  out[c, m] = sum_n v[c, n] * attn[m, n]            [1024, 1024]

Device mapping (data-parallel, 2 batches per core on 8 cores):
  * q,k computed as ONE composite 4x4 stride-2 convolution of x (the 3x3
    dc conv and 2x2 proj convs fold on the host) with 96 output channels.
    The conv consumes x as 4 stride-2 parity planes per half (stride-1
    innermost free dim); each plane is used ONCE (both m-half psums
    accumulate in parallel on 2 banks) so plane DMAs pace the pipe only
    at their arrival rate.
  * startup: wqk weight-group and x-plane DMAs are interleaved in exact
    consumption order, staggered across the two HWDGE queues (SP + ACT),
    so the first matmul starts ~11us in instead of ~23us.
  * the q/k path stays fp32(r) end-to-end: score errors pass through
    exp() and get amplified ~10x, bf16 there alone costs ~1e-2 rel err.
  * the v path, attention weights E, V^T tiles and the output are bf16
    (sim: 5.1e-3 global rel err vs 2e-2 gate): halves wv/output DMA and
    SBUF, and enables fast-weight-load on the PE stationary operands.
  * fp8 DoubleRow was simulated and REJECTED: e4m3 on the v conv alone
    is 4e-2 global rel err, fp8 attn 2.4e-2 -- both over the gate.
  * scores computed transposed: T[n, m] = S[m, n], E = exp(T) (|S|<=~25
    so f32 exp is safe), U^T[m, c] = sum_n E[n, m] V^T[n, c], row sums
    D[m] via ones-matmul on the bf16-accumulated esum, out^T = U^T/D as
    per-partition scale on eviction, stored bf16, host transposes+casts.
"""
import os
import sys
import types
import numpy as np
from contextlib import ExitStack

for _p in ("/opt/trn_rl_repo", "/root/.axon_site/_ro/trn_rl_repo"):
    if os.path.isdir(_p) and _p not in sys.path:
        sys.path.insert(0, _p)

import ml_dtypes
import concourse.bacc as bacc
import concourse.bass as bass
import concourse.tile as tile
import concourse.mybir as mybir
from concourse import bass_utils

F32 = mybir.dt.float32
F32R = mybir.dt.float32r
BF16 = mybir.dt.bfloat16
ts = bass.ts

NCORES = 8
BPC = 2          # batches per core
C_IN = 256
NPOS = 1024      # 32*32 output positions


def _install_ntff_hook_shim():
    """Register the axon NTFF profile hook if the image's antenv lacks it."""
    if "antenv.axon_hooks" in sys.modules:
        return
    try:
        from trn_agent_boot.trn_boot import _ntff_profile_via_ctypes
        hook = _ntff_profile_via_ctypes("/opt/axon/libaxon_pjrt.so")
    except Exception:
        hook = None
    m = types.ModuleType("antenv.axon_hooks")
    m.get_axon_ntff_profile_hook = lambda: hook
    m.set_axon_ntff_profile_hook = lambda h: None
    sys.modules["antenv.axon_hooks"] = m


def build_program():
    """Build the per-core Bacc program (same program on all 8 cores)."""
    nc = bacc.Bacc(trn_type="TRN2", target_bir_lowering=False, debug=False)

    # padded x as 4 stride-2 parity planes: xq[b, c, a*2+p, r, s] =
    # x_pad[b, c, 2r+a, 2s+p]
    xq = nc.dram_tensor("xq", [BPC, C_IN, 4, 33, 33], F32, kind="ExternalInput")
    wqk = nc.dram_tensor("wqk", [128, 32, 96], F32, kind="ExternalInput")
    wv = nc.dram_tensor("wv", [128, 8, 1024], BF16, kind="ExternalInput")
    bqk = nc.dram_tensor("bqk", [96, 1], F32, kind="ExternalInput")
    bvr = nc.dram_tensor("bvr", [128, 1024], F32, kind="ExternalInput")
    o = nc.dram_tensor("o", [BPC, 1024, 1024], BF16, kind="ExternalOutput")

    EXP = mybir.ActivationFunctionType.Exp
    COPY = mybir.ActivationFunctionType.Copy

    with tile.TileContext(nc) as tc, ExitStack() as ctx:
        const = ctx.enter_context(tc.tile_pool(name="const", bufs=1))
        xpool = ctx.enter_context(tc.tile_pool(name="xpool", bufs=16))
        xspool = ctx.enter_context(tc.tile_pool(name="xspool", bufs=16))
        qkp = ctx.enter_context(tc.tile_pool(name="qkp", bufs=1))
        epool = ctx.enter_context(tc.tile_pool(name="epool", bufs=1))
        vtpool = ctx.enter_context(tc.tile_pool(name="vtpool", bufs=1))
        outp = ctx.enter_context(tc.tile_pool(name="outp", bufs=4))
        misc = ctx.enter_context(tc.tile_pool(name="misc", bufs=2))
        ppq = ctx.enter_context(tc.tile_pool(name="ppq", bufs=1, space="PSUM"))
        ppt = ctx.enter_context(tc.tile_pool(name="ppt", bufs=2, space="PSUM"))
        ppv = ctx.enter_context(tc.tile_pool(name="ppv", bufs=2, space="PSUM"))
        ppu = ctx.enter_context(tc.tile_pool(name="ppu", bufs=2, space="PSUM"))

        # ---- persistent constants (tiles now; DMAs interleaved below) ----
        wqk_g = [
            const.tile([128, 4, 96], F32R, tag=f"wqk_g{g}", name=f"wqk_g{g}")
            for g in range(8)
        ]
        wv_t = [
            const.tile([128, 1024], BF16, tag=f"wv_sb{ck}", name=f"wv_sb{ck}")
            for ck in range(8)
        ]
        bqk_sb = const.tile([96, 1], F32, tag="bqk_sb")
        bvr_sb = const.tile([128, 1024], F32, tag="bvr_sb")
        # N=2 f32r ones for the D-sum matmuls (memset can't write f32r)
        ones_f32 = const.tile([128, 2], F32, tag="ones_f32")
        nc.vector.memset(ones_f32[:], 1.0)
        ones2 = const.tile([128, 2], F32R, tag="ones2")
        nc.scalar.copy(ones2[:], ones_f32[:])

        for b in range(BPC):
            # ---- x parity planes + (b0) wqk groups, interleaved in the
            # exact qk-conv consumption order and staggered across the two
            # HWDGE queues so planes land every ~1.7us ----
            xh = [[None] * 4 for _ in range(2)]
            for h in range(2):
                for pl in range(4):
                    g = h * 4 + pl
                    eng = nc.sync if g % 2 == 0 else nc.scalar
                    if b == 0:
                        eng.dma_start(
                            wqk_g[g][:], wqk.ap().bitcast(F32R)[:, 4 * g : 4 * g + 4, :]
                        )
                    xt = xpool.tile([128, 33, 33], F32R, tag="xh")
                    eng.dma_start(xt[:], xq.ap().bitcast(F32R)[b, ts(h, 128), pl])
                    xh[h][pl] = xt
            if b == 0:
                nc.sync.dma_start(bqk_sb[:], bqk.ap())
                for ck in range(8):
                    nc.scalar.dma_start(wv_t[ck][:], wv.ap()[:, ck, :])
                nc.scalar.dma_start(bvr_sb[:], bvr.ap())

            # space-to-depth x for the v conv, derived ON DEVICE from the
            # parity planes, cast to bf16 (fast-weight-load stationary).
            # Ordered by plane arrival (p00, p01, p02, p03, p10, ...).
            xs_c = [None] * 8
            for ck in (6, 4, 2, 0, 7, 5, 3, 1):
                t, h = divmod(ck, 2)
                dy, dx = divmod(t, 2)
                a, u2 = (dy + 1) % 2, (dy + 1) // 2
                p2, v2 = (dx + 1) % 2, (dx + 1) // 2
                xst = xspool.tile([128, 1024], BF16, tag="xs")
                srcv = xh[h][a * 2 + p2][:, u2 : u2 + 32, v2 : v2 + 32]
                dstv = xst[:].rearrange("p (a b) -> p a b", a=32)
                # b1 casts go to gpsimd: the DVE stream must never wait on
                # b1 plane DMAs mid-b0 (head-of-line psum-release stalls)
                (nc.vector if b == 0 else nc.gpsimd).tensor_copy(dstv, srcv)
                xs_c[ck] = xst

            # ---- composite q|k conv: both m-half psums accumulate in
            # parallel so each plane is consumed once, right as it lands ----
            QK = qkp.tile([96, 1024], F32R, tag="QK")
            pq_t = [
                ppq.tile([96, 512], F32, tag=f"pq{jm}", name=f"pq{jm}")
                for jm in range(2)
            ]
            for h in range(2):
                for pl in range(4):
                    for u in range(2):
                        for v in range(2):
                            ck2 = h * 16 + pl * 4 + u * 2 + v
                            first = ck2 == 0
                            last = ck2 == 31
                            for jm in range(2):
                                rhs = xh[h][pl][
                                    :, u + 16 * jm : u + 16 * jm + 16, v : v + 32
                                ]
                                nc.tensor.matmul(
                                    pq_t[jm][:], wqk_g[ck2 // 4][:, ck2 % 4, :], rhs,
                                    start=first, stop=last,
                                )
            for jm in range(2):
                nc.vector.tensor_scalar_add(
                    QK[:, ts(jm, 512)], pq_t[jm][:], bqk_sb[:, :1]
                )
            # q rows to partition base 0 for use as the scores rhs (engine
            # accesses must start at partition 0/32/64; DMA is unrestricted)
            Qs = qkp.tile([48, 1024], F32R, tag="Qs")
            nc.gpsimd.dma_start(Qs[:], QK[48:96, :])

            # ---- v conv (V^T tiles, bf16) interleaved with the transposed
            # scores matmuls + exps + esum partials ----
            e_sb = epool.tile([128, 8, 1024], BF16, tag="e_sb")
            vt_sb = vtpool.tile([128, 8, 1024], BF16, tag="vt_sb")
            esum = epool.tile([128, 1024], F32R, tag="esum")
            for g in range(16):
                jn, l = divmod(g, 2)
                pv_t = ppv.tile([128, 512], F32, tag="pv")
                for ck in range(8):
                    nc.tensor.matmul(
                        pv_t[:], xs_c[ck][:, ts(jn, 128)], wv_t[ck][:, ts(l, 512)],
                        start=(ck == 0), stop=(ck == 7),
                    )
                nc.vector.tensor_add(
                    vt_sb[:, jn, ts(l, 512)], pv_t[:], bvr_sb[:, ts(l, 512)]
                )
                # scores chunk g: T[n, m] = S[m, n] for n-chunk g//2, m-half
                # g%2 (k/q evicted to separate base-0 tiles -- matmul
                # operands must share base partition 0/32/64)
                sn, sm = divmod(g, 2)
                pt_t = ppt.tile([128, 512], F32, tag="pt")
                nc.tensor.matmul(
                    pt_t[:], QK[0:48, ts(sn, 128)], Qs[:, ts(sm, 512)],
                    start=True, stop=True,
                )
                nc.scalar.activation(e_sb[:, sn, ts(sm, 512)], pt_t[:], EXP)
                if g % 2 == 1:
                    if sn == 1:
                        nc.gpsimd.tensor_add(esum[:], e_sb[:, 0, :], e_sb[:, 1, :])
                    elif sn > 1:
                        nc.gpsimd.tensor_add(esum[:], esum[:], e_sb[:, sn, :])

            # ---- U^T[m, c] = sum_n E[n, m] V^T[n, c]; D[m]; out^T = U^T/D ----
            for mm in range(8):
                pd_t = ppv.tile([128, 2], F32, tag="pv", name="pd_t")
                nc.tensor.matmul(
                    pd_t[:], esum[:, ts(mm, 128)], ones2[:], start=True, stop=True
                )
                rc = misc.tile([128, 1], F32, tag="rc")
                nc.vector.reciprocal(rc[:], pd_t[:, 0:1])
                for l in range(2):
                    pu_t = ppu.tile([128, 512], F32, tag="pu")
                    for jn in range(8):
                        nc.tensor.matmul(
                            pu_t[:], e_sb[:, jn, ts(mm, 128)], vt_sb[:, jn, ts(l, 512)],
                            start=(jn == 0), stop=(jn == 7),
                        )
                    ot = outp.tile([128, 512], BF16, tag="ot")
                    nc.scalar.activation(ot[:], pu_t[:], COPY, scale=rc[:])
                    nc.gpsimd.dma_start(o.ap()[b, ts(mm, 128), ts(l, 512)], ot[:])

    nc.compile()
    return nc


def host_weights(dc_w, dc_b, q_w, k_w, q_b, k_b, v_w, v_b):
    """Fold dc conv into q/k projections -> composite 4x4 stride-2 weights."""
    dc_w = np.asarray(dc_w, np.float32)
    dc_b = np.asarray(dc_b, np.float32)
    q_w = np.asarray(q_w, np.float32)
    k_w = np.asarray(k_w, np.float32)
    q_b = np.asarray(q_b, np.float32)
    k_b = np.asarray(k_b, np.float32)
    v_w = np.asarray(v_w, np.float32)
    v_b = np.asarray(v_b, np.float32)

    C = dc_w.shape[1]
    Wq = np.zeros((48, C, 4, 4), np.float64)
    Wk = np.zeros((48, C, 4, 4), np.float64)
    for p in range(2):
        for qq in range(2):
            qw_pq = q_w[:, :, p, qq].astype(np.float64)
            kw_pq = k_w[:, :, p, qq].astype(np.float64)
            for dy in range(3):
                for dx in range(3):
                    dcw_dd = dc_w[:, :, dy, dx].astype(np.float64)
                    Wq[:, :, p + dy, qq + dx] += qw_pq @ dcw_dd
                    Wk[:, :, p + dy, qq + dx] += kw_pq @ dcw_dd
    bq_eff = q_b + q_w.sum(axis=(2, 3)) @ dc_b
    bk_eff = k_b + k_w.sum(axis=(2, 3)) @ dc_b
    # lhsT row index = (A*4+B)*C + c', columns: k 0:48 | q 48:96
    wqk_ab = (
        np.concatenate(
            [
                Wk.transpose(2, 3, 1, 0).reshape(16 * C, 48),
                Wq.transpose(2, 3, 1, 0).reshape(16 * C, 48),
            ],
            axis=1,
        )
        .astype(np.float32)
        .reshape(32, 128, 96)  # chunk_old = (A*4+B)*2 + h
    )
    # permute chunks into device consumption order (h, pl, u, v)
    perm = []
    for h in range(2):
        for pl in range(4):
            a, p = divmod(pl, 2)
            for u in range(2):
                for v in range(2):
                    A, Bo = 2 * u + a, 2 * v + p
                    perm.append((A * 4 + Bo) * 2 + h)
    wqk = wqk_ab[perm].transpose(1, 0, 2)  # [part 128, chunk2 32, 96]
    bqk = np.concatenate([bk_eff, bq_eff]).reshape(96, 1).astype(np.float32)
    # v rhs: row = (dy*2+dx)*C + c', col = oc -- sent bf16
    wv = np.ascontiguousarray(
        v_w.transpose(2, 3, 1, 0).reshape(8, 128, 4 * C).transpose(1, 0, 2)
    ).astype(ml_dtypes.bfloat16)  # [part 128, chunk 8, oc]
    bvr = np.ascontiguousarray(np.broadcast_to(v_b, (128, 4 * C))).astype(np.float32)
    return wqk, bqk, wv, bvr


_PROGRAM = None
LAST_RESULTS = None


def _get_program():
    global _PROGRAM
    if _PROGRAM is None:
        _PROGRAM = build_program()
    return _PROGRAM


def kernel(x, dc_w, dc_b, q_w, q_b, k_w, k_b, v_w, v_b):
    _install_ntff_hook_shim()
    x = np.asarray(x, np.float32)
    B = x.shape[0]
    xp = np.pad(x, ((0, 0), (0, 0), (1, 1), (1, 1)))
    # parity planes: xq[b, c, a*2+p, r, s] = x_pad[b, c, 2r+a, 2s+p]
    xq = (
        xp.reshape(B, C_IN, 33, 2, 33, 2)
        .transpose(0, 1, 3, 5, 2, 4)
        .reshape(B, C_IN, 4, 33, 33)
    )
    wqk, bqk, wv, bvr = host_weights(dc_w, dc_b, q_w, k_w, q_b, k_b, v_w, v_b)

    nc = _get_program()
    in_maps = []
    for c in range(NCORES):
        in_maps.append(
            {
                "xq": np.ascontiguousarray(xq[BPC * c : BPC * (c + 1)]),
                "wqk": wqk,
                "wv": wv,
                "bqk": bqk,
                "bvr": bvr,
            }
        )
    res = bass_utils.run_bass_kernel_spmd(nc, in_maps, core_ids=list(range(NCORES)))
    global LAST_RESULTS
    LAST_RESULTS = res

    out = np.empty((B, 1024, 1024), np.float32)
    for c in range(NCORES):
        out[BPC * c : BPC * (c + 1)] = (
            np.asarray(res.results[c]["o"]).astype(np.float32).transpose(0, 2, 1)
        )
    return out


# revision 10
# speedup vs baseline: 1.1643x; 1.0290x over previous
"""Trainium2 Bass kernel for nn_DGM_77318001263213 (dense_transformer).

Reference computation (per batch b of 16):
  dir_map = conv3x3_SAME(x, dc_w) + dc_b            [12, 64, 64]
  q = conv2x2_s2(dir_map, q_w) + q_b  -> [48, 1024]
  k = conv2x2_s2(dir_map, k_w) + k_b  -> [48, 1024]
  v = conv2x2_s2(x, v_w) + v_b        -> [1024, 1024]
  attn = softmax(q^T k, axis=-1)                    [1024, 1024]
  out[c, m] = sum_n v[c, n] * attn[m, n]            [1024, 1024]

Device mapping (data-parallel, 2 batches per core on 8 cores):
  * q,k computed as ONE composite 4x4 stride-2 convolution of x (the 3x3
    dc conv and 2x2 proj convs fold on the host) with 96 output channels.
    The conv consumes x as 4 stride-2 parity planes per half (stride-1
    innermost free dim); each plane is used ONCE (both m-half psums
    accumulate in parallel on 2 banks) so plane DMAs pace the pipe only
    at their arrival rate.
  * startup: wqk weight-group and x-plane DMAs are interleaved in exact
    consumption order, staggered across the two HWDGE queues (SP + ACT),
    so the first matmul starts ~11us in instead of ~23us.
  * the q/k path stays fp32(r) end-to-end: score errors pass through
    exp() and get amplified ~10x, bf16 there alone costs ~1e-2 rel err.
  * the v path, attention weights E, V^T tiles and the output are bf16
    (sim: 5.1e-3 global rel err vs 2e-2 gate): halves wv/output DMA and
    SBUF, and enables fast-weight-load on the PE stationary operands.
  * fp8 DoubleRow was simulated and REJECTED: e4m3 on the v conv alone
    is 4e-2 global rel err, fp8 attn 2.4e-2 -- both over the gate.
  * scores computed transposed: T[n, m] = S[m, n], E = exp(T) (|S|<=~25
    so f32 exp is safe), U^T[m, c] = sum_n E[n, m] V^T[n, c], row sums
    D[m] via ones-matmul on the bf16-accumulated esum, out^T = U^T/D as
    per-partition scale on eviction, stored bf16, host transposes+casts.
"""
import os
import sys
import types
import numpy as np
from contextlib import ExitStack

for _p in ("/opt/trn_rl_repo", "/root/.axon_site/_ro/trn_rl_repo"):
    if os.path.isdir(_p) and _p not in sys.path:
        sys.path.insert(0, _p)

import ml_dtypes
import concourse.bacc as bacc
import concourse.bass as bass
import concourse.tile as tile
import concourse.mybir as mybir
from concourse import bass_utils

F32 = mybir.dt.float32
F32R = mybir.dt.float32r
BF16 = mybir.dt.bfloat16
ts = bass.ts

NCORES = 8
BPC = 2          # batches per core
C_IN = 256
NPOS = 1024      # 32*32 output positions


def _install_ntff_hook_shim():
    """Register the axon NTFF profile hook if the image's antenv lacks it."""
    if "antenv.axon_hooks" in sys.modules:
        return
    try:
        from trn_agent_boot.trn_boot import _ntff_profile_via_ctypes
        hook = _ntff_profile_via_ctypes("/opt/axon/libaxon_pjrt.so")
    except Exception:
        hook = None
    m = types.ModuleType("antenv.axon_hooks")
    m.get_axon_ntff_profile_hook = lambda: hook
    m.set_axon_ntff_profile_hook = lambda h: None
    sys.modules["antenv.axon_hooks"] = m


def build_program():
    """Build the per-core Bacc program (same program on all 8 cores)."""
    nc = bacc.Bacc(trn_type="TRN2", target_bir_lowering=False, debug=False)

    # padded x as 4 stride-2 parity planes: xq[b, c, a*2+p, r, s] =
    # x_pad[b, c, 2r+a, 2s+p]
    xq = nc.dram_tensor("xq", [BPC, C_IN, 4, 33, 33], F32, kind="ExternalInput")
    wqk = nc.dram_tensor("wqk", [128, 32, 96], F32, kind="ExternalInput")
    wv = nc.dram_tensor("wv", [128, 8, 1024], BF16, kind="ExternalInput")
    bqk = nc.dram_tensor("bqk", [96, 1], F32, kind="ExternalInput")
    bvr = nc.dram_tensor("bvr", [128, 1024], F32, kind="ExternalInput")
    o = nc.dram_tensor("o", [BPC, 1024, 1024], BF16, kind="ExternalOutput")

    EXP = mybir.ActivationFunctionType.Exp
    COPY = mybir.ActivationFunctionType.Copy

    with tile.TileContext(nc) as tc, ExitStack() as ctx:
        const = ctx.enter_context(tc.tile_pool(name="const", bufs=1))
        xpool = ctx.enter_context(tc.tile_pool(name="xpool", bufs=16))
        xspool = ctx.enter_context(tc.tile_pool(name="xspool", bufs=16))
        qkp = ctx.enter_context(tc.tile_pool(name="qkp", bufs=1))
        epool = ctx.enter_context(tc.tile_pool(name="epool", bufs=1))
        vtpool = ctx.enter_context(tc.tile_pool(name="vtpool", bufs=1))
        outp = ctx.enter_context(tc.tile_pool(name="outp", bufs=4))
        misc = ctx.enter_context(tc.tile_pool(name="misc", bufs=2))
        ppq = ctx.enter_context(tc.tile_pool(name="ppq", bufs=1, space="PSUM"))
        ppt = ctx.enter_context(tc.tile_pool(name="ppt", bufs=2, space="PSUM"))
        ppv = ctx.enter_context(tc.tile_pool(name="ppv", bufs=2, space="PSUM"))
        ppu = ctx.enter_context(tc.tile_pool(name="ppu", bufs=2, space="PSUM"))

        # ---- persistent constants (tiles now; DMAs interleaved below) ----
        wqk_g = [
            const.tile([128, 4, 96], F32R, tag=f"wqk_g{g}", name=f"wqk_g{g}")
            for g in range(8)
        ]
        wv_t = [
            const.tile([128, 1024], BF16, tag=f"wv_sb{ck}", name=f"wv_sb{ck}")
            for ck in range(8)
        ]
        bqk_sb = const.tile([96, 1], F32, tag="bqk_sb")
        bvr_sb = const.tile([128, 1024], F32, tag="bvr_sb")
        # N=2 f32r ones for the D-sum matmuls (memset can't write f32r)
        ones_f32 = const.tile([128, 2], F32, tag="ones_f32")
        nc.vector.memset(ones_f32[:], 1.0)
        ones2 = const.tile([128, 2], F32R, tag="ones2")
        nc.scalar.copy(ones2[:], ones_f32[:])

        # ---- phase A: ALL input DMAs + space-to-depth casts for BOTH
        # batches up front.  b0's wqk groups and planes interleave in exact
        # consumption order, staggered across the two HWDGE queues; b1's
        # planes stream behind them while b0 computes.  b1's casts run on
        # gpsimd (idle until b0's output stores ~60us in) so the DVE stream
        # never waits on b1 plane DMAs mid-b0. ----
        xh_all, xs_all = [], []
        for b in range(BPC):
            xh = [[None] * 4 for _ in range(2)]
            for h in range(2):
                for pl in range(4):
                    g = h * 4 + pl
                    eng = nc.sync if g % 2 == 0 else nc.scalar
                    if b == 0:
                        eng.dma_start(
                            wqk_g[g][:], wqk.ap().bitcast(F32R)[:, 4 * g : 4 * g + 4, :]
                        )
                    xt = xpool.tile([128, 33, 33], F32R, tag="xh")
                    eng.dma_start(xt[:], xq.ap().bitcast(F32R)[b, ts(h, 128), pl])
                    xh[h][pl] = xt
            xh_all.append(xh)
            if b == 0:
                nc.sync.dma_start(bqk_sb[:], bqk.ap())
                for ck in range(8):
                    nc.scalar.dma_start(wv_t[ck][:], wv.ap()[:, ck, :])
                nc.scalar.dma_start(bvr_sb[:], bvr.ap())
        for b in range(BPC):
            xs_c = [None] * 8
            for ck in (6, 4, 2, 0, 7, 5, 3, 1):
                t, h = divmod(ck, 2)
                dy, dx = divmod(t, 2)
                a, u2 = (dy + 1) % 2, (dy + 1) // 2
                p2, v2 = (dx + 1) % 2, (dx + 1) // 2
                xst = xspool.tile([128, 1024], BF16, tag="xs")
                srcv = xh_all[b][h][a * 2 + p2][:, u2 : u2 + 32, v2 : v2 + 32]
                dstv = xst[:].rearrange("p (a b) -> p a b", a=32)
                (nc.vector if b == 0 else nc.gpsimd).tensor_copy(dstv, srcv)
                xs_c[ck] = xst
            xs_all.append(xs_c)

        for b in range(BPC):
            xh = xh_all[b]
            xs_c = xs_all[b]
            # ---- composite q|k conv: both m-half psums accumulate in
            # parallel so each plane is consumed once, right as it lands ----
            QK = qkp.tile([96, 1024], F32R, tag="QK")
            pq_t = [
                ppq.tile([96, 512], F32, tag=f"pq{jm}", name=f"pq{jm}")
                for jm in range(2)
            ]
            for h in range(2):
                for pl in range(4):
                    for u in range(2):
                        for v in range(2):
                            ck2 = h * 16 + pl * 4 + u * 2 + v
                            first = ck2 == 0
                            last = ck2 == 31
                            for jm in range(2):
                                rhs = xh[h][pl][
                                    :, u + 16 * jm : u + 16 * jm + 16, v : v + 32
                                ]
                                nc.tensor.matmul(
                                    pq_t[jm][:], wqk_g[ck2 // 4][:, ck2 % 4, :], rhs,
                                    start=first, stop=last,
                                )
            # q rows to partition base 0 for use as the scores rhs (engine
            # accesses must start at partition 0/32/64; DMA is unrestricted).
            # Each m-half is copied right after its eviction so the first
            # scores matmul isn't gated on the second half.
            Qs = qkp.tile([48, 1024], F32R, tag="Qs")
            for jm in range(2):
                nc.vector.tensor_scalar_add(
                    QK[:, ts(jm, 512)], pq_t[jm][:], bqk_sb[:, :1]
                )
                nc.gpsimd.dma_start(Qs[:, ts(jm, 512)], QK[48:96, ts(jm, 512)])

            # ---- v conv (V^T tiles, bf16) interleaved with the transposed
            # scores matmuls + exps + esum partials ----
            e_sb = epool.tile([128, 8, 1024], BF16, tag="e_sb")
            vt_sb = vtpool.tile([128, 8, 1024], BF16, tag="vt_sb")
            esum = epool.tile([128, 1024], F32R, tag="esum")
            for g in range(16):
                jn, l = divmod(g, 2)
                pv_t = ppv.tile([128, 512], F32, tag="pv")
                for ck in range(8):
                    nc.tensor.matmul(
                        pv_t[:], xs_c[ck][:, ts(jn, 128)], wv_t[ck][:, ts(l, 512)],
                        start=(ck == 0), stop=(ck == 7),
                    )
                nc.vector.tensor_add(
                    vt_sb[:, jn, ts(l, 512)], pv_t[:], bvr_sb[:, ts(l, 512)]
                )
                # scores chunk g: T[n, m] = S[m, n] for n-chunk g//2, m-half
                # g%2 (k/q evicted to separate base-0 tiles -- matmul
                # operands must share base partition 0/32/64)
                sn, sm = divmod(g, 2)
                pt_t = ppt.tile([128, 512], F32, tag="pt")
                nc.tensor.matmul(
                    pt_t[:], QK[0:48, ts(sn, 128)], Qs[:, ts(sm, 512)],
                    start=True, stop=True,
                )
                nc.scalar.activation(e_sb[:, sn, ts(sm, 512)], pt_t[:], EXP)
                if g % 2 == 1:
                    if sn == 1:
                        nc.vector.tensor_add(esum[:], e_sb[:, 0, :], e_sb[:, 1, :])
                    elif sn > 1:
                        nc.vector.tensor_add(esum[:], esum[:], e_sb[:, sn, :])

            # ---- U^T[m, c] = sum_n E[n, m] V^T[n, c]; D[m]; out^T = U^T/D ----
            for mm in range(8):
                pd_t = ppv.tile([128, 2], F32, tag="pv", name="pd_t")
                nc.tensor.matmul(
                    pd_t[:], esum[:, ts(mm, 128)], ones2[:], start=True, stop=True
                )
                rc = misc.tile([128, 1], F32, tag="rc")
                nc.vector.reciprocal(rc[:], pd_t[:, 0:1])
                for l in range(2):
                    pu_t = ppu.tile([128, 512], F32, tag="pu")
                    for jn in range(8):
                        nc.tensor.matmul(
                            pu_t[:], e_sb[:, jn, ts(mm, 128)], vt_sb[:, jn, ts(l, 512)],
                            start=(jn == 0), stop=(jn == 7),
                        )
                    ot = outp.tile([128, 512], BF16, tag="ot")
                    nc.scalar.activation(ot[:], pu_t[:], COPY, scale=rc[:])
                    nc.gpsimd.dma_start(o.ap()[b, ts(mm, 128), ts(l, 512)], ot[:])

    nc.compile()
    return nc


def host_weights(dc_w, dc_b, q_w, k_w, q_b, k_b, v_w, v_b):
    """Fold dc conv into q/k projections -> composite 4x4 stride-2 weights."""
    dc_w = np.asarray(dc_w, np.float32)
    dc_b = np.asarray(dc_b, np.float32)
    q_w = np.asarray(q_w, np.float32)
    k_w = np.asarray(k_w, np.float32)
    q_b = np.asarray(q_b, np.float32)
    k_b = np.asarray(k_b, np.float32)
    v_w = np.asarray(v_w, np.float32)
    v_b = np.asarray(v_b, np.float32)

    C = dc_w.shape[1]
    Wq = np.zeros((48, C, 4, 4), np.float64)
    Wk = np.zeros((48, C, 4, 4), np.float64)
    for p in range(2):
        for qq in range(2):
            qw_pq = q_w[:, :, p, qq].astype(np.float64)
            kw_pq = k_w[:, :, p, qq].astype(np.float64)
            for dy in range(3):
                for dx in range(3):
                    dcw_dd = dc_w[:, :, dy, dx].astype(np.float64)
                    Wq[:, :, p + dy, qq + dx] += qw_pq @ dcw_dd
                    Wk[:, :, p + dy, qq + dx] += kw_pq @ dcw_dd
    bq_eff = q_b + q_w.sum(axis=(2, 3)) @ dc_b
    bk_eff = k_b + k_w.sum(axis=(2, 3)) @ dc_b
    # lhsT row index = (A*4+B)*C + c', columns: k 0:48 | q 48:96
    wqk_ab = (
        np.concatenate(
            [
                Wk.transpose(2, 3, 1, 0).reshape(16 * C, 48),
                Wq.transpose(2, 3, 1, 0).reshape(16 * C, 48),
            ],
            axis=1,
        )
        .astype(np.float32)
        .reshape(32, 128, 96)  # chunk_old = (A*4+B)*2 + h
    )
    # permute chunks into device consumption order (h, pl, u, v)
    perm = []
    for h in range(2):
        for pl in range(4):
            a, p = divmod(pl, 2)
            for u in range(2):
                for v in range(2):
                    A, Bo = 2 * u + a, 2 * v + p
                    perm.append((A * 4 + Bo) * 2 + h)
    wqk = wqk_ab[perm].transpose(1, 0, 2)  # [part 128, chunk2 32, 96]
    bqk = np.concatenate([bk_eff, bq_eff]).reshape(96, 1).astype(np.float32)
    # v rhs: row = (dy*2+dx)*C + c', col = oc -- sent bf16
    wv = np.ascontiguousarray(
        v_w.transpose(2, 3, 1, 0).reshape(8, 128, 4 * C).transpose(1, 0, 2)
    ).astype(ml_dtypes.bfloat16)  # [part 128, chunk 8, oc]
    bvr = np.ascontiguousarray(np.broadcast_to(v_b, (128, 4 * C))).astype(np.float32)
    return wqk, bqk, wv, bvr


_PROGRAM = None
LAST_RESULTS = None


def _get_program():
    global _PROGRAM
    if _PROGRAM is None:
        _PROGRAM = build_program()
    return _PROGRAM


def kernel(x, dc_w, dc_b, q_w, q_b, k_w, k_b, v_w, v_b):
    _install_ntff_hook_shim()
    x = np.asarray(x, np.float32)
    B = x.shape[0]
    xp = np.pad(x, ((0, 0), (0, 0), (1, 1), (1, 1)))
    # parity planes: xq[b, c, a*2+p, r, s] = x_pad[b, c, 2r+a, 2s+p]
    xq = (
        xp.reshape(B, C_IN, 33, 2, 33, 2)
        .transpose(0, 1, 3, 5, 2, 4)
        .reshape(B, C_IN, 4, 33, 33)
    )
    wqk, bqk, wv, bvr = host_weights(dc_w, dc_b, q_w, k_w, q_b, k_b, v_w, v_b)

    nc = _get_program()
    in_maps = []
    for c in range(NCORES):
        in_maps.append(
            {
                "xq": np.ascontiguousarray(xq[BPC * c : BPC * (c + 1)]),
                "wqk": wqk,
                "wv": wv,
                "bqk": bqk,
                "bvr": bvr,
            }
        )
    res = bass_utils.run_bass_kernel_spmd(nc, in_maps, core_ids=list(range(NCORES)))
    global LAST_RESULTS
    LAST_RESULTS = res

    out = np.empty((B, 1024, 1024), np.float32)
    for c in range(NCORES):
        out[BPC * c : BPC * (c + 1)] = (
            np.asarray(res.results[c]["o"]).astype(np.float32).transpose(0, 2, 1)
        )
    return out


# revision 11
# speedup vs baseline: 1.2292x; 1.0557x over previous
"""Trainium2 Bass kernel for nn_DGM_77318001263213 (dense_transformer).

Reference computation (per batch b of 16):
  dir_map = conv3x3_SAME(x, dc_w) + dc_b            [12, 64, 64]
  q = conv2x2_s2(dir_map, q_w) + q_b  -> [48, 1024]
  k = conv2x2_s2(dir_map, k_w) + k_b  -> [48, 1024]
  v = conv2x2_s2(x, v_w) + v_b        -> [1024, 1024]
  attn = softmax(q^T k, axis=-1)                    [1024, 1024]
  out[c, m] = sum_n v[c, n] * attn[m, n]            [1024, 1024]

Device mapping (data-parallel, 2 batches per core on 8 cores):
  * q,k computed as ONE composite 4x4 stride-2 convolution of x (the 3x3
    dc conv and 2x2 proj convs fold on the host) with 96 output channels.
    The conv consumes x as 4 stride-2 parity planes per half (stride-1
    innermost free dim); each plane is used ONCE (both m-half psums
    accumulate in parallel on 2 banks) so plane DMAs pace the pipe only
    at their arrival rate.
  * startup: wqk weight-group and x-plane DMAs are interleaved in exact
    consumption order, staggered across the two HWDGE queues (SP + ACT),
    so the first matmul starts ~11us in instead of ~23us.
  * the q/k path stays fp32(r) end-to-end: score errors pass through
    exp() and get amplified ~10x, bf16 there alone costs ~1e-2 rel err.
  * the v path, attention weights E, V^T tiles and the output are bf16
    (sim: 5.1e-3 global rel err vs 2e-2 gate): halves wv/output DMA and
    SBUF, and enables fast-weight-load on the PE stationary operands.
  * fp8 DoubleRow was simulated and REJECTED: e4m3 on the v conv alone
    is 4e-2 global rel err, fp8 attn 2.4e-2 -- both over the gate.
  * scores computed transposed: T[n, m] = S[m, n], E = exp(T) (|S|<=~25
    so f32 exp is safe), U^T[m, c] = sum_n E[n, m] V^T[n, c], row sums
    D[m] via ones-matmul on the bf16-accumulated esum, out^T = U^T/D as
    per-partition scale on eviction, stored bf16, host transposes+casts.
"""
import os
import sys
import types
import numpy as np
from contextlib import ExitStack

for _p in ("/opt/trn_rl_repo", "/root/.axon_site/_ro/trn_rl_repo"):
    if os.path.isdir(_p) and _p not in sys.path:
        sys.path.insert(0, _p)

import ml_dtypes
import concourse.bacc as bacc
import concourse.bass as bass
import concourse.tile as tile
import concourse.mybir as mybir
from concourse import bass_utils

F32 = mybir.dt.float32
F32R = mybir.dt.float32r
BF16 = mybir.dt.bfloat16
ts = bass.ts

NCORES = 8
BPC = 2          # batches per core
C_IN = 256
NPOS = 1024      # 32*32 output positions


def _install_ntff_hook_shim():
    """Register the axon NTFF profile hook if the image's antenv lacks it."""
    if "antenv.axon_hooks" in sys.modules:
        return
    try:
        from trn_agent_boot.trn_boot import _ntff_profile_via_ctypes
        hook = _ntff_profile_via_ctypes("/opt/axon/libaxon_pjrt.so")
    except Exception:
        hook = None
    m = types.ModuleType("antenv.axon_hooks")
    m.get_axon_ntff_profile_hook = lambda: hook
    m.set_axon_ntff_profile_hook = lambda h: None
    sys.modules["antenv.axon_hooks"] = m


def build_program():
    """Build the per-core Bacc program (same program on all 8 cores)."""
    nc = bacc.Bacc(trn_type="TRN2", target_bir_lowering=False, debug=False)

    # padded x as 4 stride-2 parity planes: xq[b, c, a*2+p, r, s] =
    # x_pad[b, c, 2r+a, 2s+p]
    xq = nc.dram_tensor("xq", [BPC, C_IN, 4, 33, 33], F32, kind="ExternalInput")
    wqk = nc.dram_tensor("wqk", [128, 32, 112], F32, kind="ExternalInput")
    wv = nc.dram_tensor("wv", [128, 8, 1024], BF16, kind="ExternalInput")
    bqk = nc.dram_tensor("bqk", [112, 1], F32, kind="ExternalInput")
    bvr = nc.dram_tensor("bvr", [128, 1024], F32, kind="ExternalInput")
    o = nc.dram_tensor("o", [BPC, 1024, 1024], BF16, kind="ExternalOutput")

    EXP = mybir.ActivationFunctionType.Exp
    COPY = mybir.ActivationFunctionType.Copy

    with tile.TileContext(nc) as tc, ExitStack() as ctx:
        const = ctx.enter_context(tc.tile_pool(name="const", bufs=1))
        xpool = ctx.enter_context(tc.tile_pool(name="xpool", bufs=16))
        xspool = ctx.enter_context(tc.tile_pool(name="xspool", bufs=16))
        qkp = ctx.enter_context(tc.tile_pool(name="qkp", bufs=1))
        epool = ctx.enter_context(tc.tile_pool(name="epool", bufs=1))
        vtpool = ctx.enter_context(tc.tile_pool(name="vtpool", bufs=1))
        outp = ctx.enter_context(tc.tile_pool(name="outp", bufs=4))
        misc = ctx.enter_context(tc.tile_pool(name="misc", bufs=2))
        ppq = ctx.enter_context(tc.tile_pool(name="ppq", bufs=1, space="PSUM"))
        ppt = ctx.enter_context(tc.tile_pool(name="ppt", bufs=2, space="PSUM"))
        ppv = ctx.enter_context(tc.tile_pool(name="ppv", bufs=2, space="PSUM"))
        ppu = ctx.enter_context(tc.tile_pool(name="ppu", bufs=2, space="PSUM"))

        # ---- persistent constants (tiles now; DMAs interleaved below) ----
        wqk_g = [
            const.tile([128, 4, 112], F32R, tag=f"wqk_g{g}", name=f"wqk_g{g}")
            for g in range(8)
        ]
        wv_t = [
            const.tile([128, 1024], BF16, tag=f"wv_sb{ck}", name=f"wv_sb{ck}")
            for ck in range(8)
        ]
        bqk_sb = const.tile([112, 1], F32, tag="bqk_sb")
        bvr_sb = const.tile([128, 1024], F32, tag="bvr_sb")
        # N=2 f32r ones for the D-sum matmuls (memset can't write f32r)
        ones_f32 = const.tile([128, 2], F32, tag="ones_f32")
        nc.vector.memset(ones_f32[:], 1.0)
        ones2 = const.tile([128, 2], F32R, tag="ones2")
        nc.scalar.copy(ones2[:], ones_f32[:])

        # ---- phase A: ALL input DMAs + space-to-depth casts for BOTH
        # batches up front.  b0's wqk groups and planes interleave in exact
        # consumption order, staggered across the two HWDGE queues; b1's
        # planes stream behind them while b0 computes.  b1's casts run on
        # gpsimd (idle until b0's output stores ~60us in) so the DVE stream
        # never waits on b1 plane DMAs mid-b0. ----
        xh_all, xs_all = [], []
        for b in range(BPC):
            xh = [[None] * 4 for _ in range(2)]
            for h in range(2):
                for pl in range(4):
                    g = h * 4 + pl
                    eng = nc.sync if g % 2 == 0 else nc.scalar
                    if b == 0:
                        eng.dma_start(
                            wqk_g[g][:], wqk.ap().bitcast(F32R)[:, 4 * g : 4 * g + 4, :]
                        )
                    xt = xpool.tile([128, 33, 33], F32R, tag="xh")
                    eng.dma_start(xt[:], xq.ap().bitcast(F32R)[b, ts(h, 128), pl])
                    xh[h][pl] = xt
            xh_all.append(xh)
            if b == 0:
                nc.sync.dma_start(bqk_sb[:], bqk.ap())
                for ck in range(8):
                    nc.scalar.dma_start(wv_t[ck][:], wv.ap()[:, ck, :])
                nc.scalar.dma_start(bvr_sb[:], bvr.ap())
        for b in range(BPC):
            xs_c = [None] * 8
            for ck in (6, 4, 2, 0, 7, 5, 3, 1):
                t, h = divmod(ck, 2)
                dy, dx = divmod(t, 2)
                a, u2 = (dy + 1) % 2, (dy + 1) // 2
                p2, v2 = (dx + 1) % 2, (dx + 1) // 2
                xst = xspool.tile([128, 1024], BF16, tag="xs")
                srcv = xh_all[b][h][a * 2 + p2][:, u2 : u2 + 32, v2 : v2 + 32]
                dstv = xst[:].rearrange("p (a b) -> p a b", a=32)
                (nc.vector if b == 0 else nc.gpsimd).tensor_copy(dstv, srcv)
                xs_c[ck] = xst
            xs_all.append(xs_c)

        for b in range(BPC):
            xh = xh_all[b]
            xs_c = xs_all[b]
            # ---- composite q|k conv: both m-half psums accumulate in
            # parallel so each plane is consumed once, right as it lands ----
            # k rows sit at psum partitions 0:48, q at 64:112 (weight cols
            # 48:64 are zero-padded) so BOTH evict straight to base-0 tiles
            # (engine partition bases must be 0/32/64) -- no Qs move at all.
            Ks = qkp.tile([48, 1024], F32R, tag="Ks")
            Qs = qkp.tile([48, 1024], F32R, tag="Qs")
            pq_t = [
                ppq.tile([112, 512], F32, tag=f"pq{jm}", name=f"pq{jm}")
                for jm in range(2)
            ]
            for h in range(2):
                for pl in range(4):
                    for u in range(2):
                        for v in range(2):
                            ck2 = h * 16 + pl * 4 + u * 2 + v
                            first = ck2 == 0
                            last = ck2 == 31
                            for jm in range(2):
                                rhs = xh[h][pl][
                                    :, u + 16 * jm : u + 16 * jm + 16, v : v + 32
                                ]
                                nc.tensor.matmul(
                                    pq_t[jm][:], wqk_g[ck2 // 4][:, ck2 % 4, :], rhs,
                                    start=first, stop=last,
                                )
            for jm in range(2):
                nc.vector.tensor_scalar_add(
                    Ks[:, ts(jm, 512)], pq_t[jm][0:48, :], bqk_sb[0:48, :1]
                )
                nc.vector.tensor_scalar_add(
                    Qs[:, ts(jm, 512)], pq_t[jm][64:112, :], bqk_sb[64:112, :1]
                )

            # ---- v conv (V^T tiles, bf16) interleaved with the transposed
            # scores matmuls + exps + esum partials ----
            e_sb = epool.tile([128, 8, 1024], BF16, tag="e_sb")
            vt_sb = vtpool.tile([128, 8, 1024], BF16, tag="vt_sb")
            esum = epool.tile([128, 1024], F32R, tag="esum")
            for g in range(16):
                jn, l = divmod(g, 2)
                pv_t = ppv.tile([128, 512], F32, tag="pv")
                for ck in range(8):
                    nc.tensor.matmul(
                        pv_t[:], xs_c[ck][:, ts(jn, 128)], wv_t[ck][:, ts(l, 512)],
                        start=(ck == 0), stop=(ck == 7),
                    )
                nc.vector.tensor_add(
                    vt_sb[:, jn, ts(l, 512)], pv_t[:], bvr_sb[:, ts(l, 512)]
                )
                # scores chunk g: T[n, m] = S[m, n] for n-chunk g//2, m-half
                # g%2 (k/q evicted to separate base-0 tiles -- matmul
                # operands must share base partition 0/32/64)
                sn, sm = divmod(g, 2)
                pt_t = ppt.tile([128, 512], F32, tag="pt")
                nc.tensor.matmul(
                    pt_t[:], Ks[:, ts(sn, 128)], Qs[:, ts(sm, 512)],
                    start=True, stop=True,
                )
                nc.scalar.activation(e_sb[:, sn, ts(sm, 512)], pt_t[:], EXP)
                if g % 2 == 1:
                    if sn == 1:
                        nc.vector.tensor_add(esum[:], e_sb[:, 0, :], e_sb[:, 1, :])
                    elif sn > 1:
                        nc.vector.tensor_add(esum[:], esum[:], e_sb[:, sn, :])

            # ---- U^T[m, c] = sum_n E[n, m] V^T[n, c]; D[m]; out^T = U^T/D ----
            for mm in range(8):
                pd_t = ppv.tile([128, 2], F32, tag="pv", name="pd_t")
                nc.tensor.matmul(
                    pd_t[:], esum[:, ts(mm, 128)], ones2[:], start=True, stop=True
                )
                rc = misc.tile([128, 1], F32, tag="rc")
                nc.vector.reciprocal(rc[:], pd_t[:, 0:1])
                for l in range(2):
                    pu_t = ppu.tile([128, 512], F32, tag="pu")
                    for jn in range(8):
                        nc.tensor.matmul(
                            pu_t[:], e_sb[:, jn, ts(mm, 128)], vt_sb[:, jn, ts(l, 512)],
                            start=(jn == 0), stop=(jn == 7),
                        )
                    ot = outp.tile([128, 512], BF16, tag="ot")
                    nc.scalar.activation(ot[:], pu_t[:], COPY, scale=rc[:])
                    nc.gpsimd.dma_start(o.ap()[b, ts(mm, 128), ts(l, 512)], ot[:])

    nc.compile()
    return nc


def host_weights(dc_w, dc_b, q_w, k_w, q_b, k_b, v_w, v_b):
    """Fold dc conv into q/k projections -> composite 4x4 stride-2 weights."""
    dc_w = np.asarray(dc_w, np.float32)
    dc_b = np.asarray(dc_b, np.float32)
    q_w = np.asarray(q_w, np.float32)
    k_w = np.asarray(k_w, np.float32)
    q_b = np.asarray(q_b, np.float32)
    k_b = np.asarray(k_b, np.float32)
    v_w = np.asarray(v_w, np.float32)
    v_b = np.asarray(v_b, np.float32)

    C = dc_w.shape[1]
    Wq = np.zeros((48, C, 4, 4), np.float64)
    Wk = np.zeros((48, C, 4, 4), np.float64)
    for p in range(2):
        for qq in range(2):
            qw_pq = q_w[:, :, p, qq].astype(np.float64)
            kw_pq = k_w[:, :, p, qq].astype(np.float64)
            for dy in range(3):
                for dx in range(3):
                    dcw_dd = dc_w[:, :, dy, dx].astype(np.float64)
                    Wq[:, :, p + dy, qq + dx] += qw_pq @ dcw_dd
                    Wk[:, :, p + dy, qq + dx] += kw_pq @ dcw_dd
    bq_eff = q_b + q_w.sum(axis=(2, 3)) @ dc_b
    bk_eff = k_b + k_w.sum(axis=(2, 3)) @ dc_b
    # lhsT row index = (A*4+B)*C + c', columns: k 0:48 | q 48:96
    wqk_ab = (
        np.concatenate(
            [
                Wk.transpose(2, 3, 1, 0).reshape(16 * C, 48),
                np.zeros((16 * C, 16), np.float64),
                Wq.transpose(2, 3, 1, 0).reshape(16 * C, 48),
            ],
            axis=1,
        )
        .astype(np.float32)
        .reshape(32, 128, 112)  # chunk_old = (A*4+B)*2 + h; cols 48:64 zero
    )
    # permute chunks into device consumption order (h, pl, u, v)
    perm = []
    for h in range(2):
        for pl in range(4):
            a, p = divmod(pl, 2)
            for u in range(2):
                for v in range(2):
                    A, Bo = 2 * u + a, 2 * v + p
                    perm.append((A * 4 + Bo) * 2 + h)
    wqk = wqk_ab[perm].transpose(1, 0, 2)  # [part 128, chunk2 32, 96]
    bqk = np.concatenate(
        [bk_eff, np.zeros(16, np.float64), bq_eff]
    ).reshape(112, 1).astype(np.float32)
    # v rhs: row = (dy*2+dx)*C + c', col = oc -- sent bf16
    wv = np.ascontiguousarray(
        v_w.transpose(2, 3, 1, 0).reshape(8, 128, 4 * C).transpose(1, 0, 2)
    ).astype(ml_dtypes.bfloat16)  # [part 128, chunk 8, oc]
    bvr = np.ascontiguousarray(np.broadcast_to(v_b, (128, 4 * C))).astype(np.float32)
    return wqk, bqk, wv, bvr


_PROGRAM = None
LAST_RESULTS = None


def _get_program():
    global _PROGRAM
    if _PROGRAM is None:
        _PROGRAM = build_program()
    return _PROGRAM


def kernel(x, dc_w, dc_b, q_w, q_b, k_w, k_b, v_w, v_b):
    _install_ntff_hook_shim()
    x = np.asarray(x, np.float32)
    B = x.shape[0]
    xp = np.pad(x, ((0, 0), (0, 0), (1, 1), (1, 1)))
    # parity planes: xq[b, c, a*2+p, r, s] = x_pad[b, c, 2r+a, 2s+p]
    xq = (
        xp.reshape(B, C_IN, 33, 2, 33, 2)
        .transpose(0, 1, 3, 5, 2, 4)
        .reshape(B, C_IN, 4, 33, 33)
    )
    wqk, bqk, wv, bvr = host_weights(dc_w, dc_b, q_w, k_w, q_b, k_b, v_w, v_b)

    nc = _get_program()
    in_maps = []
    for c in range(NCORES):
        in_maps.append(
            {
                "xq": np.ascontiguousarray(xq[BPC * c : BPC * (c + 1)]),
                "wqk": wqk,
                "wv": wv,
                "bqk": bqk,
                "bvr": bvr,
            }
        )
    res = bass_utils.run_bass_kernel_spmd(nc, in_maps, core_ids=list(range(NCORES)))
    global LAST_RESULTS
    LAST_RESULTS = res

    out = np.empty((B, 1024, 1024), np.float32)
    for c in range(NCORES):
        out[BPC * c : BPC * (c + 1)] = (
            np.asarray(res.results[c]["o"]).astype(np.float32).transpose(0, 2, 1)
        )
    return out


# revision 12
# speedup vs baseline: 1.2409x; 1.0095x over previous
"""Trainium2 Bass kernel for nn_DGM_77318001263213 (dense_transformer).

Reference computation (per batch b of 16):
  dir_map = conv3x3_SAME(x, dc_w) + dc_b            [12, 64, 64]
  q = conv2x2_s2(dir_map, q_w) + q_b  -> [48, 1024]
  k = conv2x2_s2(dir_map, k_w) + k_b  -> [48, 1024]
  v = conv2x2_s2(x, v_w) + v_b        -> [1024, 1024]
  attn = softmax(q^T k, axis=-1)                    [1024, 1024]
  out[c, m] = sum_n v[c, n] * attn[m, n]            [1024, 1024]

Device mapping (data-parallel, 2 batches per core on 8 cores):
  * q,k computed as ONE composite 4x4 stride-2 convolution of x (the 3x3
    dc conv and 2x2 proj convs fold on the host) with 96 output channels.
    The conv consumes x as 4 stride-2 parity planes per half (stride-1
    innermost free dim); each plane is used ONCE (both m-half psums
    accumulate in parallel on 2 banks) so plane DMAs pace the pipe only
    at their arrival rate.
  * startup: wqk weight-group and x-plane DMAs are interleaved in exact
    consumption order, staggered across the two HWDGE queues (SP + ACT),
    so the first matmul starts ~11us in instead of ~23us.
  * the q/k path stays fp32(r) end-to-end: score errors pass through
    exp() and get amplified ~10x, bf16 there alone costs ~1e-2 rel err.
  * the v path, attention weights E, V^T tiles and the output are bf16
    (sim: 5.1e-3 global rel err vs 2e-2 gate): halves wv/output DMA and
    SBUF, and enables fast-weight-load on the PE stationary operands.
  * fp8 DoubleRow was simulated and REJECTED: e4m3 on the v conv alone
    is 4e-2 global rel err, fp8 attn 2.4e-2 -- both over the gate.
  * scores computed transposed: T[n, m] = S[m, n], E = exp(T) (|S|<=~25
    so f32 exp is safe), U^T[m, c] = sum_n E[n, m] V^T[n, c], row sums
    D[m] via ones-matmul on the bf16-accumulated esum, out^T = U^T/D as
    per-partition scale on eviction, stored bf16, host transposes+casts.
"""
import os
import sys
import types
import numpy as np
from contextlib import ExitStack

for _p in ("/opt/trn_rl_repo", "/root/.axon_site/_ro/trn_rl_repo"):
    if os.path.isdir(_p) and _p not in sys.path:
        sys.path.insert(0, _p)

import ml_dtypes
import concourse.bacc as bacc
import concourse.bass as bass
import concourse.tile as tile
import concourse.mybir as mybir
from concourse import bass_utils

F32 = mybir.dt.float32
F32R = mybir.dt.float32r
BF16 = mybir.dt.bfloat16
ts = bass.ts

NCORES = 8
BPC = 2          # batches per core
C_IN = 256
NPOS = 1024      # 32*32 output positions


def _install_ntff_hook_shim():
    """Register the axon NTFF profile hook if the image's antenv lacks it."""
    if "antenv.axon_hooks" in sys.modules:
        return
    try:
        from trn_agent_boot.trn_boot import _ntff_profile_via_ctypes
        hook = _ntff_profile_via_ctypes("/opt/axon/libaxon_pjrt.so")
    except Exception:
        hook = None
    m = types.ModuleType("antenv.axon_hooks")
    m.get_axon_ntff_profile_hook = lambda: hook
    m.set_axon_ntff_profile_hook = lambda h: None
    sys.modules["antenv.axon_hooks"] = m


def build_program():
    """Build the per-core Bacc program (same program on all 8 cores)."""
    nc = bacc.Bacc(trn_type="TRN2", target_bir_lowering=False, debug=False)

    # padded x as 4 stride-2 parity planes: xq[b, c, a*2+p, r, s] =
    # x_pad[b, c, 2r+a, 2s+p]
    xq = nc.dram_tensor("xq", [BPC, C_IN, 4, 33, 33], F32, kind="ExternalInput")
    wqk = nc.dram_tensor("wqk", [128, 32, 112], F32, kind="ExternalInput")
    wv = nc.dram_tensor("wv", [128, 8, 1024], BF16, kind="ExternalInput")
    bqk = nc.dram_tensor("bqk", [112, 1], F32, kind="ExternalInput")
    bvr = nc.dram_tensor("bvr", [128, 1024], F32, kind="ExternalInput")
    o = nc.dram_tensor("o", [BPC, 1024, 1024], BF16, kind="ExternalOutput")

    EXP = mybir.ActivationFunctionType.Exp
    COPY = mybir.ActivationFunctionType.Copy

    with tile.TileContext(nc) as tc, ExitStack() as ctx:
        const = ctx.enter_context(tc.tile_pool(name="const", bufs=1))
        xpool = ctx.enter_context(tc.tile_pool(name="xpool", bufs=16))
        xspool = ctx.enter_context(tc.tile_pool(name="xspool", bufs=16))
        qkp = ctx.enter_context(tc.tile_pool(name="qkp", bufs=1))
        epool = ctx.enter_context(tc.tile_pool(name="epool", bufs=1))
        vtpool = ctx.enter_context(tc.tile_pool(name="vtpool", bufs=1))
        outp = ctx.enter_context(tc.tile_pool(name="outp", bufs=4))
        misc = ctx.enter_context(tc.tile_pool(name="misc", bufs=2))
        ppq = ctx.enter_context(tc.tile_pool(name="ppq", bufs=1, space="PSUM"))
        ppt = ctx.enter_context(tc.tile_pool(name="ppt", bufs=2, space="PSUM"))
        ppv = ctx.enter_context(tc.tile_pool(name="ppv", bufs=2, space="PSUM"))
        ppu = ctx.enter_context(tc.tile_pool(name="ppu", bufs=2, space="PSUM"))

        # ---- persistent constants (tiles now; DMAs interleaved below) ----
        wqk_g = [
            const.tile([128, 4, 112], F32R, tag=f"wqk_g{g}", name=f"wqk_g{g}")
            for g in range(8)
        ]
        wv_t = [
            const.tile([128, 1024], BF16, tag=f"wv_sb{ck}", name=f"wv_sb{ck}")
            for ck in range(8)
        ]
        bqk_sb = const.tile([112, 1], F32, tag="bqk_sb")
        bvr_sb = const.tile([128, 1024], F32, tag="bvr_sb")
        # N=2 f32r ones for the D-sum matmuls (memset can't write f32r)
        ones_f32 = const.tile([128, 2], F32, tag="ones_f32")
        nc.vector.memset(ones_f32[:], 1.0)
        ones2 = const.tile([128, 2], F32R, tag="ones2")
        nc.scalar.copy(ones2[:], ones_f32[:])

        # ---- phase A: ALL input DMAs + space-to-depth casts for BOTH
        # batches up front.  b0's wqk groups and planes interleave in exact
        # consumption order, staggered across the two HWDGE queues; b1's
        # planes stream behind them while b0 computes.  b1's casts run on
        # gpsimd (idle until b0's output stores ~60us in) so the DVE stream
        # never waits on b1 plane DMAs mid-b0. ----
        xh_all, xs_all = [], []
        for b in range(BPC):
            xh = [[None] * 4 for _ in range(2)]
            for h in range(2):
                for pl in range(4):
                    g = h * 4 + pl
                    eng = nc.sync if g % 2 == 0 else nc.scalar
                    if b == 0:
                        eng.dma_start(
                            wqk_g[g][:], wqk.ap().bitcast(F32R)[:, 4 * g : 4 * g + 4, :]
                        )
                    xt = xpool.tile([128, 33, 33], F32R, tag="xh")
                    eng.dma_start(xt[:], xq.ap().bitcast(F32R)[b, ts(h, 128), pl])
                    xh[h][pl] = xt
            xh_all.append(xh)
            if b == 0:
                nc.sync.dma_start(bqk_sb[:], bqk.ap())
                for ck in range(8):
                    nc.scalar.dma_start(wv_t[ck][:], wv.ap()[:, ck, :])
                nc.scalar.dma_start(bvr_sb[:], bvr.ap())
        for b in range(BPC):
            xs_c = [None] * 8
            for ck in (6, 4, 2, 0, 7, 5, 3, 1):
                t, h = divmod(ck, 2)
                dy, dx = divmod(t, 2)
                a, u2 = (dy + 1) % 2, (dy + 1) // 2
                p2, v2 = (dx + 1) % 2, (dx + 1) // 2
                xst = xspool.tile([128, 1024], BF16, tag="xs")
                srcv = xh_all[b][h][a * 2 + p2][:, u2 : u2 + 32, v2 : v2 + 32]
                dstv = xst[:].rearrange("p (a b) -> p a b", a=32)
                (nc.vector if b == 0 else nc.gpsimd).tensor_copy(dstv, srcv)
                xs_c[ck] = xst
            xs_all.append(xs_c)

        for b in range(BPC):
            xh = xh_all[b]
            xs_c = xs_all[b]
            # ---- composite q|k conv: both m-half psums accumulate in
            # parallel so each plane is consumed once, right as it lands ----
            # k rows sit at psum partitions 0:48, q at 64:112 (weight cols
            # 48:64 are zero-padded) so BOTH evict straight to base-0 tiles
            # (engine partition bases must be 0/32/64) -- no Qs move at all.
            Ks = qkp.tile([48, 1024], BF16, tag="Ks")
            Qs = qkp.tile([48, 1024], BF16, tag="Qs")
            pq_t = [
                ppq.tile([112, 512], F32, tag=f"pq{jm}", name=f"pq{jm}")
                for jm in range(2)
            ]
            for h in range(2):
                for pl in range(4):
                    for u in range(2):
                        for v in range(2):
                            ck2 = h * 16 + pl * 4 + u * 2 + v
                            first = ck2 == 0
                            last = ck2 == 31
                            for jm in range(2):
                                rhs = xh[h][pl][
                                    :, u + 16 * jm : u + 16 * jm + 16, v : v + 32
                                ]
                                nc.tensor.matmul(
                                    pq_t[jm][:], wqk_g[ck2 // 4][:, ck2 % 4, :], rhs,
                                    start=first, stop=last,
                                )
            for jm in range(2):
                nc.vector.tensor_scalar_add(
                    Ks[:, ts(jm, 512)], pq_t[jm][0:48, :], bqk_sb[0:48, :1]
                )
                nc.vector.tensor_scalar_add(
                    Qs[:, ts(jm, 512)], pq_t[jm][64:112, :], bqk_sb[64:112, :1]
                )

            # ---- v conv (V^T tiles, bf16) interleaved with the transposed
            # scores matmuls + exps + esum partials ----
            e_sb = epool.tile([128, 8, 1024], BF16, tag="e_sb")
            vt_sb = vtpool.tile([128, 8, 1024], BF16, tag="vt_sb")
            esum = epool.tile([128, 1024], F32R, tag="esum")
            for g in range(16):
                jn, l = divmod(g, 2)
                pv_t = ppv.tile([128, 512], F32, tag="pv")
                for ck in range(8):
                    nc.tensor.matmul(
                        pv_t[:], xs_c[ck][:, ts(jn, 128)], wv_t[ck][:, ts(l, 512)],
                        start=(ck == 0), stop=(ck == 7),
                    )
                nc.vector.tensor_add(
                    vt_sb[:, jn, ts(l, 512)], pv_t[:], bvr_sb[:, ts(l, 512)]
                )
                # scores chunk g: T[n, m] = S[m, n] for n-chunk g//2, m-half
                # g%2 (k/q evicted to separate base-0 tiles -- matmul
                # operands must share base partition 0/32/64)
                sn, sm = divmod(g, 2)
                pt_t = ppt.tile([128, 512], F32, tag="pt")
                nc.tensor.matmul(
                    pt_t[:], Ks[:, ts(sn, 128)], Qs[:, ts(sm, 512)],
                    start=True, stop=True,
                )
                nc.scalar.activation(e_sb[:, sn, ts(sm, 512)], pt_t[:], EXP)
                if g % 2 == 1:
                    if sn == 1:
                        nc.vector.tensor_add(esum[:], e_sb[:, 0, :], e_sb[:, 1, :])
                    elif sn > 1:
                        nc.vector.tensor_add(esum[:], esum[:], e_sb[:, sn, :])

            # ---- U^T[m, c] = sum_n E[n, m] V^T[n, c]; D[m]; out^T = U^T/D ----
            for mm in range(8):
                pd_t = ppv.tile([128, 2], F32, tag="pv", name="pd_t")
                nc.tensor.matmul(
                    pd_t[:], esum[:, ts(mm, 128)], ones2[:], start=True, stop=True
                )
                rc = misc.tile([128, 1], F32, tag="rc")
                nc.vector.reciprocal(rc[:], pd_t[:, 0:1])
                for l in range(2):
                    pu_t = ppu.tile([128, 512], F32, tag="pu")
                    for jn in range(8):
                        nc.tensor.matmul(
                            pu_t[:], e_sb[:, jn, ts(mm, 128)], vt_sb[:, jn, ts(l, 512)],
                            start=(jn == 0), stop=(jn == 7),
                        )
                    ot = outp.tile([128, 512], BF16, tag="ot")
                    nc.scalar.activation(ot[:], pu_t[:], COPY, scale=rc[:])
                    nc.gpsimd.dma_start(o.ap()[b, ts(mm, 128), ts(l, 512)], ot[:])

    nc.compile()
    return nc


def host_weights(dc_w, dc_b, q_w, k_w, q_b, k_b, v_w, v_b):
    """Fold dc conv into q/k projections -> composite 4x4 stride-2 weights."""
    dc_w = np.asarray(dc_w, np.float32)
    dc_b = np.asarray(dc_b, np.float32)
    q_w = np.asarray(q_w, np.float32)
    k_w = np.asarray(k_w, np.float32)
    q_b = np.asarray(q_b, np.float32)
    k_b = np.asarray(k_b, np.float32)
    v_w = np.asarray(v_w, np.float32)
    v_b = np.asarray(v_b, np.float32)

    C = dc_w.shape[1]
    Wq = np.zeros((48, C, 4, 4), np.float64)
    Wk = np.zeros((48, C, 4, 4), np.float64)
    for p in range(2):
        for qq in range(2):
            qw_pq = q_w[:, :, p, qq].astype(np.float64)
            kw_pq = k_w[:, :, p, qq].astype(np.float64)
            for dy in range(3):
                for dx in range(3):
                    dcw_dd = dc_w[:, :, dy, dx].astype(np.float64)
                    Wq[:, :, p + dy, qq + dx] += qw_pq @ dcw_dd
                    Wk[:, :, p + dy, qq + dx] += kw_pq @ dcw_dd
    bq_eff = q_b + q_w.sum(axis=(2, 3)) @ dc_b
    bk_eff = k_b + k_w.sum(axis=(2, 3)) @ dc_b
    # lhsT row index = (A*4+B)*C + c', columns: k 0:48 | q 48:96
    wqk_ab = (
        np.concatenate(
            [
                Wk.transpose(2, 3, 1, 0).reshape(16 * C, 48),
                np.zeros((16 * C, 16), np.float64),
                Wq.transpose(2, 3, 1, 0).reshape(16 * C, 48),
            ],
            axis=1,
        )
        .astype(np.float32)
        .reshape(32, 128, 112)  # chunk_old = (A*4+B)*2 + h; cols 48:64 zero
    )
    # permute chunks into device consumption order (h, pl, u, v)
    perm = []
    for h in range(2):
        for pl in range(4):
            a, p = divmod(pl, 2)
            for u in range(2):
                for v in range(2):
                    A, Bo = 2 * u + a, 2 * v + p
                    perm.append((A * 4 + Bo) * 2 + h)
    wqk = wqk_ab[perm].transpose(1, 0, 2)  # [part 128, chunk2 32, 96]
    bqk = np.concatenate(
        [bk_eff, np.zeros(16, np.float64), bq_eff]
    ).reshape(112, 1).astype(np.float32)
    # v rhs: row = (dy*2+dx)*C + c', col = oc -- sent bf16
    wv = np.ascontiguousarray(
        v_w.transpose(2, 3, 1, 0).reshape(8, 128, 4 * C).transpose(1, 0, 2)
    ).astype(ml_dtypes.bfloat16)  # [part 128, chunk 8, oc]
    bvr = np.ascontiguousarray(np.broadcast_to(v_b, (128, 4 * C))).astype(np.float32)
    return wqk, bqk, wv, bvr


_PROGRAM = None
LAST_RESULTS = None


def _get_program():
    global _PROGRAM
    if _PROGRAM is None:
        _PROGRAM = build_program()
    return _PROGRAM


def kernel(x, dc_w, dc_b, q_w, q_b, k_w, k_b, v_w, v_b):
    _install_ntff_hook_shim()
    x = np.asarray(x, np.float32)
    B = x.shape[0]
    xp = np.pad(x, ((0, 0), (0, 0), (1, 1), (1, 1)))
    # parity planes: xq[b, c, a*2+p, r, s] = x_pad[b, c, 2r+a, 2s+p]
    xq = (
        xp.reshape(B, C_IN, 33, 2, 33, 2)
        .transpose(0, 1, 3, 5, 2, 4)
        .reshape(B, C_IN, 4, 33, 33)
    )
    wqk, bqk, wv, bvr = host_weights(dc_w, dc_b, q_w, k_w, q_b, k_b, v_w, v_b)

    nc = _get_program()
    in_maps = []
    for c in range(NCORES):
        in_maps.append(
            {
                "xq": np.ascontiguousarray(xq[BPC * c : BPC * (c + 1)]),
                "wqk": wqk,
                "wv": wv,
                "bqk": bqk,
                "bvr": bvr,
            }
        )
    res = bass_utils.run_bass_kernel_spmd(nc, in_maps, core_ids=list(range(NCORES)))
    global LAST_RESULTS
    LAST_RESULTS = res

    out = np.empty((B, 1024, 1024), np.float32)
    for c in range(NCORES):
        out[BPC * c : BPC * (c + 1)] = (
            np.asarray(res.results[c]["o"]).astype(np.float32).transpose(0, 2, 1)
        )
    return out


# revision 14
# speedup vs baseline: 1.2530x; 1.0098x over previous
"""Trainium2 Bass kernel for nn_DGM_77318001263213 (dense_transformer).

Reference computation (per batch b of 16):
  dir_map = conv3x3_SAME(x, dc_w) + dc_b            [12, 64, 64]
  q = conv2x2_s2(dir_map, q_w) + q_b  -> [48, 1024]
  k = conv2x2_s2(dir_map, k_w) + k_b  -> [48, 1024]
  v = conv2x2_s2(x, v_w) + v_b        -> [1024, 1024]
  attn = softmax(q^T k, axis=-1)                    [1024, 1024]
  out[c, m] = sum_n v[c, n] * attn[m, n]            [1024, 1024]

Device mapping (data-parallel, 2 batches per core on 8 cores):
  * q,k computed as ONE composite 4x4 stride-2 convolution of x (the 3x3
    dc conv and 2x2 proj convs fold on the host) with 96 output channels.
    The conv consumes x as 4 stride-2 parity planes per half (stride-1
    innermost free dim); each plane is used ONCE (both m-half psums
    accumulate in parallel on 2 banks) so plane DMAs pace the pipe only
    at their arrival rate.
  * startup: wqk weight-group and x-plane DMAs are interleaved in exact
    consumption order, staggered across the two HWDGE queues (SP + ACT),
    so the first matmul starts ~11us in instead of ~23us.
  * the q/k path stays fp32(r) end-to-end: score errors pass through
    exp() and get amplified ~10x, bf16 there alone costs ~1e-2 rel err.
  * the v path, attention weights E, V^T tiles and the output are bf16
    (sim: 5.1e-3 global rel err vs 2e-2 gate): halves wv/output DMA and
    SBUF, and enables fast-weight-load on the PE stationary operands.
  * fp8 DoubleRow was simulated and REJECTED: e4m3 on the v conv alone
    is 4e-2 global rel err, fp8 attn 2.4e-2 -- both over the gate.
  * scores computed transposed: T[n, m] = S[m, n], E = exp(T) (|S|<=~25
    so f32 exp is safe), U^T[m, c] = sum_n E[n, m] V^T[n, c], row sums
    D[m] via ones-matmul on the bf16-accumulated esum, out^T = U^T/D as
    per-partition scale on eviction, stored bf16, host transposes+casts.
"""
import os
import sys
import types
import numpy as np
from contextlib import ExitStack

for _p in ("/opt/trn_rl_repo", "/root/.axon_site/_ro/trn_rl_repo"):
    if os.path.isdir(_p) and _p not in sys.path:
        sys.path.insert(0, _p)

import ml_dtypes
import concourse.bacc as bacc
import concourse.bass as bass
import concourse.tile as tile
import concourse.mybir as mybir
from concourse import bass_utils

F32 = mybir.dt.float32
F32R = mybir.dt.float32r
BF16 = mybir.dt.bfloat16
ts = bass.ts

NCORES = 8
BPC = 2          # batches per core
C_IN = 256
NPOS = 1024      # 32*32 output positions


def _install_ntff_hook_shim():
    """Register the axon NTFF profile hook if the image's antenv lacks it."""
    if "antenv.axon_hooks" in sys.modules:
        return
    try:
        from trn_agent_boot.trn_boot import _ntff_profile_via_ctypes
        hook = _ntff_profile_via_ctypes("/opt/axon/libaxon_pjrt.so")
    except Exception:
        hook = None
    m = types.ModuleType("antenv.axon_hooks")
    m.get_axon_ntff_profile_hook = lambda: hook
    m.set_axon_ntff_profile_hook = lambda h: None
    sys.modules["antenv.axon_hooks"] = m


def build_program():
    """Build the per-core Bacc program (same program on all 8 cores)."""
    nc = bacc.Bacc(trn_type="TRN2", target_bir_lowering=False, debug=False)

    # padded x as 4 stride-2 parity planes: xq[b, c, a*2+p, r, s] =
    # x_pad[b, c, 2r+a, 2s+p]
    xq = nc.dram_tensor("xq", [BPC, C_IN, 4, 33, 33], F32, kind="ExternalInput")
    wqk = nc.dram_tensor("wqk", [128, 32, 112], F32, kind="ExternalInput")
    wv = nc.dram_tensor("wv", [128, 8, 1024], BF16, kind="ExternalInput")
    bqk = nc.dram_tensor("bqk", [112, 1], F32, kind="ExternalInput")
    bvr = nc.dram_tensor("bvr", [128, 1024], F32, kind="ExternalInput")
    o = nc.dram_tensor("o", [BPC, 1024, 1024], BF16, kind="ExternalOutput")

    EXP = mybir.ActivationFunctionType.Exp
    COPY = mybir.ActivationFunctionType.Copy

    with tile.TileContext(nc) as tc, ExitStack() as ctx:
        const = ctx.enter_context(tc.tile_pool(name="const", bufs=1))
        xpool = ctx.enter_context(tc.tile_pool(name="xpool", bufs=16))
        xspool = ctx.enter_context(tc.tile_pool(name="xspool", bufs=16))
        qkp = ctx.enter_context(tc.tile_pool(name="qkp", bufs=1))
        epool = ctx.enter_context(tc.tile_pool(name="epool", bufs=1))
        vtpool = ctx.enter_context(tc.tile_pool(name="vtpool", bufs=1))
        outp = ctx.enter_context(tc.tile_pool(name="outp", bufs=4))
        misc = ctx.enter_context(tc.tile_pool(name="misc", bufs=2))
        ppq = ctx.enter_context(tc.tile_pool(name="ppq", bufs=1, space="PSUM"))
        ppt = ctx.enter_context(tc.tile_pool(name="ppt", bufs=2, space="PSUM"))
        ppv = ctx.enter_context(tc.tile_pool(name="ppv", bufs=2, space="PSUM"))
        ppu = ctx.enter_context(tc.tile_pool(name="ppu", bufs=2, space="PSUM"))

        # ---- persistent constants (tiles now; DMAs interleaved below) ----
        wqk_g = [
            const.tile([128, 4, 112], F32R, tag=f"wqk_g{g}", name=f"wqk_g{g}")
            for g in range(8)
        ]
        wv_t = [
            const.tile([128, 1024], BF16, tag=f"wv_sb{ck}", name=f"wv_sb{ck}")
            for ck in range(8)
        ]
        bqk_sb = const.tile([112, 1], F32, tag="bqk_sb")
        bvr_sb = const.tile([128, 1024], F32, tag="bvr_sb")
        # N=2 f32r ones for the D-sum matmuls (memset can't write f32r)
        ones_f32 = const.tile([128, 2], F32, tag="ones_f32")
        nc.vector.memset(ones_f32[:], 1.0)
        ones2 = const.tile([128, 2], F32R, tag="ones2")
        nc.scalar.copy(ones2[:], ones_f32[:])

        # ---- phase A: ALL input DMAs + space-to-depth casts for BOTH
        # batches up front.  b0's wqk groups and planes interleave in exact
        # consumption order, staggered across the two HWDGE queues; b1's
        # planes stream behind them while b0 computes.  b1's casts run on
        # gpsimd (idle until b0's output stores ~60us in) so the DVE stream
        # never waits on b1 plane DMAs mid-b0. ----
        xh_all, xs_all = [], []
        for b in range(BPC):
            xh = [[None] * 4 for _ in range(2)]
            for h in range(2):
                for pl in range(4):
                    g = h * 4 + pl
                    eng = nc.sync if g % 2 == 0 else nc.scalar
                    wsrc = wqk.ap().bitcast(F32R)
                    if b == 0 and g == 0:
                        eng.dma_start(wqk_g[0][:, 0:1, :], wsrc[:, 0:1, :])
                        eng.dma_start(wqk_g[0][:, 1:4, :], wsrc[:, 1:4, :])
                    elif b == 0:
                        eng.dma_start(wqk_g[g][:], wsrc[:, 4 * g : 4 * g + 4, :])
                    xt = xpool.tile([128, 33, 33], F32R, tag="xh")
                    xsrc = xq.ap().bitcast(F32R)[b, ts(h, 128), pl]
                    if b == 0 and g == 0:
                        eng.dma_start(xt[:, 0:17, :], xsrc[:, 0:17, :])
                        eng.dma_start(xt[:, 17:33, :], xsrc[:, 17:33, :])
                    else:
                        eng.dma_start(xt[:], xsrc)
                    xh[h][pl] = xt
            xh_all.append(xh)
            if b == 0:
                nc.sync.dma_start(bqk_sb[:], bqk.ap())
                for ck in range(8):
                    nc.scalar.dma_start(wv_t[ck][:], wv.ap()[:, ck, :])
                nc.scalar.dma_start(bvr_sb[:], bvr.ap())
        for b in range(BPC):
            xs_c = [None] * 8
            for ck in (6, 4, 2, 0, 7, 5, 3, 1):
                t, h = divmod(ck, 2)
                dy, dx = divmod(t, 2)
                a, u2 = (dy + 1) % 2, (dy + 1) // 2
                p2, v2 = (dx + 1) % 2, (dx + 1) // 2
                xst = xspool.tile([128, 1024], BF16, tag="xs")
                srcv = xh_all[b][h][a * 2 + p2][:, u2 : u2 + 32, v2 : v2 + 32]
                dstv = xst[:].rearrange("p (a b) -> p a b", a=32)
                (nc.vector if b == 0 else nc.gpsimd).tensor_copy(dstv, srcv)
                xs_c[ck] = xst
            xs_all.append(xs_c)

        for b in range(BPC):
            xh = xh_all[b]
            xs_c = xs_all[b]
            # ---- composite q|k conv: both m-half psums accumulate in
            # parallel so each plane is consumed once, right as it lands ----
            # k rows sit at psum partitions 0:48, q at 64:112 (weight cols
            # 48:64 are zero-padded) so BOTH evict straight to base-0 tiles
            # (engine partition bases must be 0/32/64) -- no Qs move at all.
            Ks = qkp.tile([48, 1024], BF16, tag="Ks")
            Qs = qkp.tile([48, 1024], BF16, tag="Qs")
            pq_t = [
                ppq.tile([112, 512], F32, tag=f"pq{jm}", name=f"pq{jm}")
                for jm in range(2)
            ]
            for h in range(2):
                for pl in range(4):
                    jm_orders = (
                        ((0, 0, 0), (0, 1, 0), (1, 0, 0), (1, 1, 0),
                         (0, 0, 1), (0, 1, 1), (1, 0, 1), (1, 1, 1))
                        if (b == 0 and h == 0 and pl == 0)
                        else tuple(
                            (u, v, jm) for u in range(2) for v in range(2)
                            for jm in range(2)
                        )
                    )
                    for u, v, jm in jm_orders:
                        ck2 = h * 16 + pl * 4 + u * 2 + v
                        rhs = xh[h][pl][
                            :, u + 16 * jm : u + 16 * jm + 16, v : v + 32
                        ]
                        nc.tensor.matmul(
                            pq_t[jm][:], wqk_g[ck2 // 4][:, ck2 % 4, :], rhs,
                            start=(ck2 == 0), stop=(ck2 == 31),
                        )
            for jm in range(2):
                nc.vector.tensor_scalar_add(
                    Ks[:, ts(jm, 512)], pq_t[jm][0:48, :], bqk_sb[0:48, :1]
                )
                nc.vector.tensor_scalar_add(
                    Qs[:, ts(jm, 512)], pq_t[jm][64:112, :], bqk_sb[64:112, :1]
                )

            # ---- v conv (V^T tiles, bf16) interleaved with the transposed
            # scores matmuls + exps + esum partials ----
            e_sb = epool.tile([128, 8, 1024], BF16, tag="e_sb")
            vt_sb = vtpool.tile([128, 8, 1024], BF16, tag="vt_sb")
            esum = epool.tile([128, 1024], F32R, tag="esum")
            for g in range(16):
                jn, l = divmod(g, 2)
                pv_t = ppv.tile([128, 512], F32, tag="pv")
                for ck in range(8):
                    nc.tensor.matmul(
                        pv_t[:], xs_c[ck][:, ts(jn, 128)], wv_t[ck][:, ts(l, 512)],
                        start=(ck == 0), stop=(ck == 7),
                    )
                nc.vector.tensor_add(
                    vt_sb[:, jn, ts(l, 512)], pv_t[:], bvr_sb[:, ts(l, 512)]
                )
                # scores chunk g: T[n, m] = S[m, n] for n-chunk g//2, m-half
                # g%2 (k/q evicted to separate base-0 tiles -- matmul
                # operands must share base partition 0/32/64)
                sn, sm = divmod(g, 2)
                pt_t = ppt.tile([128, 512], F32, tag="pt")
                nc.tensor.matmul(
                    pt_t[:], Ks[:, ts(sn, 128)], Qs[:, ts(sm, 512)],
                    start=True, stop=True,
                )
                nc.scalar.activation(e_sb[:, sn, ts(sm, 512)], pt_t[:], EXP)
                if g % 2 == 1:
                    if sn == 1:
                        nc.vector.tensor_add(esum[:], e_sb[:, 0, :], e_sb[:, 1, :])
                    elif sn > 1:
                        nc.vector.tensor_add(esum[:], esum[:], e_sb[:, sn, :])

            # ---- U^T[m, c] = sum_n E[n, m] V^T[n, c]; D[m]; out^T = U^T/D ----
            for mm in range(8):
                pd_t = ppv.tile([128, 2], F32, tag="pv", name="pd_t")
                nc.tensor.matmul(
                    pd_t[:], esum[:, ts(mm, 128)], ones2[:], start=True, stop=True
                )
                rc = misc.tile([128, 1], F32, tag="rc")
                nc.vector.reciprocal(rc[:], pd_t[:, 0:1])
                for l in range(2):
                    pu_t = ppu.tile([128, 512], F32, tag="pu")
                    for jn in range(8):
                        nc.tensor.matmul(
                            pu_t[:], e_sb[:, jn, ts(mm, 128)], vt_sb[:, jn, ts(l, 512)],
                            start=(jn == 0), stop=(jn == 7),
                        )
                    ot = outp.tile([128, 512], BF16, tag="ot")
                    nc.scalar.activation(ot[:], pu_t[:], COPY, scale=rc[:])
                    if b == BPC - 1 and mm == 7:
                        seng = nc.sync if l == 0 else nc.scalar
                    else:
                        seng = nc.gpsimd
                    seng.dma_start(o.ap()[b, ts(mm, 128), ts(l, 512)], ot[:])

    nc.compile()
    return nc


def host_weights(dc_w, dc_b, q_w, k_w, q_b, k_b, v_w, v_b):
    """Fold dc conv into q/k projections -> composite 4x4 stride-2 weights."""
    dc_w = np.asarray(dc_w, np.float32)
    dc_b = np.asarray(dc_b, np.float32)
    q_w = np.asarray(q_w, np.float32)
    k_w = np.asarray(k_w, np.float32)
    q_b = np.asarray(q_b, np.float32)
    k_b = np.asarray(k_b, np.float32)
    v_w = np.asarray(v_w, np.float32)
    v_b = np.asarray(v_b, np.float32)

    C = dc_w.shape[1]
    Wq = np.zeros((48, C, 4, 4), np.float64)
    Wk = np.zeros((48, C, 4, 4), np.float64)
    for p in range(2):
        for qq in range(2):
            qw_pq = q_w[:, :, p, qq].astype(np.float64)
            kw_pq = k_w[:, :, p, qq].astype(np.float64)
            for dy in range(3):
                for dx in range(3):
                    dcw_dd = dc_w[:, :, dy, dx].astype(np.float64)
                    Wq[:, :, p + dy, qq + dx] += qw_pq @ dcw_dd
                    Wk[:, :, p + dy, qq + dx] += kw_pq @ dcw_dd
    bq_eff = q_b + q_w.sum(axis=(2, 3)) @ dc_b
    bk_eff = k_b + k_w.sum(axis=(2, 3)) @ dc_b
    # lhsT row index = (A*4+B)*C + c', columns: k 0:48 | q 48:96
    wqk_ab = (
        np.concatenate(
            [
                Wk.transpose(2, 3, 1, 0).reshape(16 * C, 48),
                np.zeros((16 * C, 16), np.float64),
                Wq.transpose(2, 3, 1, 0).reshape(16 * C, 48),
            ],
            axis=1,
        )
        .astype(np.float32)
        .reshape(32, 128, 112)  # chunk_old = (A*4+B)*2 + h; cols 48:64 zero
    )
    # permute chunks into device consumption order (h, pl, u, v)
    perm = []
    for h in range(2):
        for pl in range(4):
            a, p = divmod(pl, 2)
            for u in range(2):
                for v in range(2):
                    A, Bo = 2 * u + a, 2 * v + p
                    perm.append((A * 4 + Bo) * 2 + h)
    wqk = wqk_ab[perm].transpose(1, 0, 2)  # [part 128, chunk2 32, 96]
    bqk = np.concatenate(
        [bk_eff, np.zeros(16, np.float64), bq_eff]
    ).reshape(112, 1).astype(np.float32)
    # v rhs: row = (dy*2+dx)*C + c', col = oc -- sent bf16
    wv = np.ascontiguousarray(
        v_w.transpose(2, 3, 1, 0).reshape(8, 128, 4 * C).transpose(1, 0, 2)
    ).astype(ml_dtypes.bfloat16)  # [part 128, chunk 8, oc]
    bvr = np.ascontiguousarray(np.broadcast_to(v_b, (128, 4 * C))).astype(np.float32)
    return wqk, bqk, wv, bvr


_PROGRAM = None
LAST_RESULTS = None


def _get_program():
    global _PROGRAM
    if _PROGRAM is None:
        _PROGRAM = build_program()
    return _PROGRAM


def kernel(x, dc_w, dc_b, q_w, q_b, k_w, k_b, v_w, v_b):
    _install_ntff_hook_shim()
    x = np.asarray(x, np.float32)
    B = x.shape[0]
    xp = np.pad(x, ((0, 0), (0, 0), (1, 1), (1, 1)))
    # parity planes: xq[b, c, a*2+p, r, s] = x_pad[b, c, 2r+a, 2s+p]
    xq = (
        xp.reshape(B, C_IN, 33, 2, 33, 2)
        .transpose(0, 1, 3, 5, 2, 4)
        .reshape(B, C_IN, 4, 33, 33)
    )
    wqk, bqk, wv, bvr = host_weights(dc_w, dc_b, q_w, k_w, q_b, k_b, v_w, v_b)

    nc = _get_program()
    in_maps = []
    for c in range(NCORES):
        in_maps.append(
            {
                "xq": np.ascontiguousarray(xq[BPC * c : BPC * (c + 1)]),
                "wqk": wqk,
                "wv": wv,
                "bqk": bqk,
                "bvr": bvr,
            }
        )
    res = bass_utils.run_bass_kernel_spmd(nc, in_maps, core_ids=list(range(NCORES)))
    global LAST_RESULTS
    LAST_RESULTS = res

    out = np.empty((B, 1024, 1024), np.float32)
    for c in range(NCORES):
        out[BPC * c : BPC * (c + 1)] = (
            np.asarray(res.results[c]["o"]).astype(np.float32).transpose(0, 2, 1)
        )
    return out


# revision 15
# speedup vs baseline: 1.2578x; 1.0038x over previous
"""Trainium2 Bass kernel for nn_DGM_77318001263213 (dense_transformer).

Reference computation (per batch b of 16):
  dir_map = conv3x3_SAME(x, dc_w) + dc_b            [12, 64, 64]
  q = conv2x2_s2(dir_map, q_w) + q_b  -> [48, 1024]
  k = conv2x2_s2(dir_map, k_w) + k_b  -> [48, 1024]
  v = conv2x2_s2(x, v_w) + v_b        -> [1024, 1024]
  attn = softmax(q^T k, axis=-1)                    [1024, 1024]
  out[c, m] = sum_n v[c, n] * attn[m, n]            [1024, 1024]

Device mapping (data-parallel, 2 batches per core on 8 cores):
  * q,k computed as ONE composite 4x4 stride-2 convolution of x (the 3x3
    dc conv and 2x2 proj convs fold on the host) with 96 output channels.
    The conv consumes x as 4 stride-2 parity planes per half (stride-1
    innermost free dim); each plane is used ONCE (both m-half psums
    accumulate in parallel on 2 banks) so plane DMAs pace the pipe only
    at their arrival rate.
  * startup: wqk weight-group and x-plane DMAs are interleaved in exact
    consumption order, staggered across the two HWDGE queues (SP + ACT),
    so the first matmul starts ~11us in instead of ~23us.
  * the q/k path stays fp32(r) end-to-end: score errors pass through
    exp() and get amplified ~10x, bf16 there alone costs ~1e-2 rel err.
  * the v path, attention weights E, V^T tiles and the output are bf16
    (sim: 5.1e-3 global rel err vs 2e-2 gate): halves wv/output DMA and
    SBUF, and enables fast-weight-load on the PE stationary operands.
  * fp8 DoubleRow was simulated and REJECTED: e4m3 on the v conv alone
    is 4e-2 global rel err, fp8 attn 2.4e-2 -- both over the gate.
  * scores computed transposed: T[n, m] = S[m, n], E = exp(T) (|S|<=~25
    so f32 exp is safe), U^T[m, c] = sum_n E[n, m] V^T[n, c], row sums
    D[m] via ones-matmul on the bf16-accumulated esum, out^T = U^T/D as
    per-partition scale on eviction, stored bf16, host transposes+casts.
"""
import os
import sys
import types
import numpy as np
from contextlib import ExitStack

for _p in ("/opt/trn_rl_repo", "/root/.axon_site/_ro/trn_rl_repo"):
    if os.path.isdir(_p) and _p not in sys.path:
        sys.path.insert(0, _p)

import ml_dtypes
import concourse.bacc as bacc
import concourse.bass as bass
import concourse.tile as tile
import concourse.mybir as mybir
from concourse import bass_utils

F32 = mybir.dt.float32
F32R = mybir.dt.float32r
BF16 = mybir.dt.bfloat16
ts = bass.ts

NCORES = 8
BPC = 2          # batches per core
C_IN = 256
NPOS = 1024      # 32*32 output positions


def _install_ntff_hook_shim():
    """Register the axon NTFF profile hook if the image's antenv lacks it."""
    if "antenv.axon_hooks" in sys.modules:
        return
    try:
        from trn_agent_boot.trn_boot import _ntff_profile_via_ctypes
        hook = _ntff_profile_via_ctypes("/opt/axon/libaxon_pjrt.so")
    except Exception:
        hook = None
    m = types.ModuleType("antenv.axon_hooks")
    m.get_axon_ntff_profile_hook = lambda: hook
    m.set_axon_ntff_profile_hook = lambda h: None
    sys.modules["antenv.axon_hooks"] = m


def build_program():
    """Build the per-core Bacc program (same program on all 8 cores)."""
    nc = bacc.Bacc(trn_type="TRN2", target_bir_lowering=False, debug=False)

    # padded x as 4 stride-2 parity planes: xq[b, c, a*2+p, r, s] =
    # x_pad[b, c, 2r+a, 2s+p]
    xq = nc.dram_tensor("xq", [BPC, C_IN, 4, 33, 33], F32, kind="ExternalInput")
    wqk = nc.dram_tensor("wqk", [128, 32, 112], F32, kind="ExternalInput")
    wv = nc.dram_tensor("wv", [128, 8, 1024], BF16, kind="ExternalInput")
    bqk = nc.dram_tensor("bqk", [112, 1], F32, kind="ExternalInput")
    bvr = nc.dram_tensor("bvr", [128, 1024], F32, kind="ExternalInput")
    o = nc.dram_tensor("o", [BPC, 1024, 1024], BF16, kind="ExternalOutput")

    EXP = mybir.ActivationFunctionType.Exp
    COPY = mybir.ActivationFunctionType.Copy

    with tile.TileContext(nc) as tc, ExitStack() as ctx:
        const = ctx.enter_context(tc.tile_pool(name="const", bufs=1))
        xpool = ctx.enter_context(tc.tile_pool(name="xpool", bufs=16))
        xspool = ctx.enter_context(tc.tile_pool(name="xspool", bufs=16))
        qkp = ctx.enter_context(tc.tile_pool(name="qkp", bufs=1))
        epool = ctx.enter_context(tc.tile_pool(name="epool", bufs=1))
        vtpool = ctx.enter_context(tc.tile_pool(name="vtpool", bufs=1))
        outp = ctx.enter_context(tc.tile_pool(name="outp", bufs=4))
        misc = ctx.enter_context(tc.tile_pool(name="misc", bufs=2))
        ppq = ctx.enter_context(tc.tile_pool(name="ppq", bufs=1, space="PSUM"))
        ppt = ctx.enter_context(tc.tile_pool(name="ppt", bufs=2, space="PSUM"))
        ppv = ctx.enter_context(tc.tile_pool(name="ppv", bufs=2, space="PSUM"))
        ppu = ctx.enter_context(tc.tile_pool(name="ppu", bufs=2, space="PSUM"))

        # ---- persistent constants (tiles now; DMAs interleaved below) ----
        wqk_g = [
            const.tile([128, 4, 112], F32R, tag=f"wqk_g{g}", name=f"wqk_g{g}")
            for g in range(8)
        ]
        wv_t = [
            const.tile([128, 1024], BF16, tag=f"wv_sb{ck}", name=f"wv_sb{ck}")
            for ck in range(8)
        ]
        bqk_sb = const.tile([112, 1], F32, tag="bqk_sb")
        bvr_sb = const.tile([128, 1024], F32, tag="bvr_sb")
        # N=2 f32r ones for the D-sum matmuls (memset can't write f32r)
        ones_f32 = const.tile([128, 2], F32, tag="ones_f32")
        nc.vector.memset(ones_f32[:], 1.0)
        ones2 = const.tile([128, 2], F32R, tag="ones2")
        nc.scalar.copy(ones2[:], ones_f32[:])

        # ---- phase A: ALL input DMAs + space-to-depth casts for BOTH
        # batches up front.  b0's wqk groups and planes interleave in exact
        # consumption order, staggered across the two HWDGE queues; b1's
        # planes stream behind them while b0 computes.  b1's casts run on
        # gpsimd (idle until b0's output stores ~60us in) so the DVE stream
        # never waits on b1 plane DMAs mid-b0. ----
        xh_all, xs_all = [], []
        for b in range(BPC):
            xh = [[None] * 4 for _ in range(2)]
            for h in range(2):
                for pl in range(4):
                    g = h * 4 + pl
                    eng = nc.sync if g % 2 == 0 else nc.scalar
                    wsrc = wqk.ap().bitcast(F32R)
                    if b == 0 and g == 0:
                        eng.dma_start(wqk_g[0][:, 0:1, :], wsrc[:, 0:1, :])
                        eng.dma_start(wqk_g[0][:, 1:4, :], wsrc[:, 1:4, :])
                    elif b == 0:
                        eng.dma_start(wqk_g[g][:], wsrc[:, 4 * g : 4 * g + 4, :])
                    xt = xpool.tile([128, 33, 33], F32R, tag="xh")
                    xsrc = xq.ap().bitcast(F32R)[b, ts(h, 128), pl]
                    if b == 0 and g == 0:
                        eng.dma_start(xt[:, 0:17, :], xsrc[:, 0:17, :])
                        eng.dma_start(xt[:, 17:33, :], xsrc[:, 17:33, :])
                    else:
                        eng.dma_start(xt[:], xsrc)
                    xh[h][pl] = xt
            xh_all.append(xh)
            if b == 0:
                nc.sync.dma_start(bqk_sb[:], bqk.ap())
                for ck in range(8):
                    nc.scalar.dma_start(wv_t[ck][:], wv.ap()[:, ck, :])
                nc.scalar.dma_start(bvr_sb[:], bvr.ap())
        for b in range(BPC):
            xs_c = [None] * 8
            for ck in (6, 4, 2, 0, 7, 5, 3, 1):
                t, h = divmod(ck, 2)
                dy, dx = divmod(t, 2)
                a, u2 = (dy + 1) % 2, (dy + 1) // 2
                p2, v2 = (dx + 1) % 2, (dx + 1) // 2
                xst = xspool.tile([128, 1024], BF16, tag="xs")
                srcv = xh_all[b][h][a * 2 + p2][:, u2 : u2 + 32, v2 : v2 + 32]
                dstv = xst[:].rearrange("p (a b) -> p a b", a=32)
                (nc.vector if b == 0 else nc.gpsimd).tensor_copy(dstv, srcv)
                xs_c[ck] = xst
            xs_all.append(xs_c)

        for b in range(BPC):
            xh = xh_all[b]
            xs_c = xs_all[b]
            # ---- composite q|k conv: both m-half psums accumulate in
            # parallel so each plane is consumed once, right as it lands ----
            # k rows sit at psum partitions 0:48, q at 64:112 (weight cols
            # 48:64 are zero-padded) so BOTH evict straight to base-0 tiles
            # (engine partition bases must be 0/32/64) -- no Qs move at all.
            Ks = qkp.tile([48, 1024], BF16, tag="Ks")
            Qs = qkp.tile([48, 1024], BF16, tag="Qs")
            pq_t = [
                ppq.tile([112, 512], F32, tag=f"pq{jm}", name=f"pq{jm}")
                for jm in range(2)
            ]
            for h in range(2):
                for pl in range(4):
                    jm_orders = (
                        ((0, 0, 0), (0, 1, 0), (1, 0, 0), (1, 1, 0),
                         (0, 0, 1), (0, 1, 1), (1, 0, 1), (1, 1, 1))
                        if (b == 0 and h == 0 and pl == 0)
                        else tuple(
                            (u, v, jm) for u in range(2) for v in range(2)
                            for jm in range(2)
                        )
                    )
                    for u, v, jm in jm_orders:
                        ck2 = h * 16 + pl * 4 + u * 2 + v
                        rhs = xh[h][pl][
                            :, u + 16 * jm : u + 16 * jm + 16, v : v + 32
                        ]
                        nc.tensor.matmul(
                            pq_t[jm][:], wqk_g[ck2 // 4][:, ck2 % 4, :], rhs,
                            start=(ck2 == 0), stop=(ck2 == 31),
                        )
            for jm in range(2):
                nc.vector.tensor_scalar_add(
                    Ks[:, ts(jm, 512)], pq_t[jm][0:48, :], bqk_sb[0:48, :1]
                )
                nc.vector.tensor_scalar_add(
                    Qs[:, ts(jm, 512)], pq_t[jm][64:112, :], bqk_sb[64:112, :1]
                )

            # ---- v conv (V^T tiles, bf16) interleaved with the transposed
            # scores matmuls + exps + esum partials ----
            e_sb = epool.tile([128, 8, 1024], BF16, tag="e_sb")
            vt_sb = vtpool.tile([128, 8, 1024], BF16, tag="vt_sb")
            esum = epool.tile([128, 1024], F32R, tag="esum")
            for g in range(16):
                jn, l = divmod(g, 2)
                pv_t = ppv.tile([128, 512], F32, tag="pv")
                for ck in range(8):
                    nc.tensor.matmul(
                        pv_t[:], xs_c[ck][:, ts(jn, 128)], wv_t[ck][:, ts(l, 512)],
                        start=(ck == 0), stop=(ck == 7),
                    )
                nc.vector.tensor_add(
                    vt_sb[:, jn, ts(l, 512)], pv_t[:], bvr_sb[:, ts(l, 512)]
                )
                # scores chunk g: T[n, m] = S[m, n] for n-chunk g//2, m-half
                # g%2 (k/q evicted to separate base-0 tiles -- matmul
                # operands must share base partition 0/32/64)
                sn, sm = divmod(g, 2)
                pt_t = ppt.tile([128, 512], F32, tag="pt")
                nc.tensor.matmul(
                    pt_t[:], Ks[:, ts(sn, 128)], Qs[:, ts(sm, 512)],
                    start=True, stop=True,
                )
                nc.scalar.activation(e_sb[:, sn, ts(sm, 512)], pt_t[:], EXP)
                if g % 2 == 1:
                    if sn == 1:
                        nc.vector.tensor_add(esum[:], e_sb[:, 0, :], e_sb[:, 1, :])
                    elif sn > 1:
                        nc.vector.tensor_add(esum[:], esum[:], e_sb[:, sn, :])

            # ---- U^T[m, c] = sum_n E[n, m] V^T[n, c]; D[m]; out^T = U^T/D ----
            for mm in range(8):
                pd_t = ppv.tile([128, 2], F32, tag="pv", name="pd_t")
                nc.tensor.matmul(
                    pd_t[:], esum[:, ts(mm, 128)], ones2[:], start=True, stop=True
                )
                rc = misc.tile([128, 1], F32, tag="rc")
                nc.vector.reciprocal(rc[:], pd_t[:, 0:1])
                for l in range(2):
                    pu_t = ppu.tile([128, 512], F32, tag="pu")
                    for jn in range(8):
                        nc.tensor.matmul(
                            pu_t[:], e_sb[:, jn, ts(mm, 128)], vt_sb[:, jn, ts(l, 512)],
                            start=(jn == 0), stop=(jn == 7),
                        )
                    ot = outp.tile([128, 512], BF16, tag="ot")
                    if b == BPC - 1 and mm == 7:
                        # final group: split evictions and fan the stores
                        # over the idle SP/ACT queues so the last HBM
                        # transfer starts as early as possible
                        half = ts(l, 512)
                        for piece, seng in ((0, nc.sync), (1, nc.scalar)):
                            pc = bass.ts(piece, 256)
                            nc.scalar.activation(
                                ot[:, pc], pu_t[:, pc], COPY, scale=rc[:]
                            )
                            seng.dma_start(
                                o.ap()[b, ts(mm, 128), l * 512 + piece * 256 :
                                       l * 512 + piece * 256 + 256],
                                ot[:, pc],
                            )
                    else:
                        nc.scalar.activation(ot[:], pu_t[:], COPY, scale=rc[:])
                        nc.gpsimd.dma_start(
                            o.ap()[b, ts(mm, 128), ts(l, 512)], ot[:]
                        )

    nc.compile()
    return nc


def host_weights(dc_w, dc_b, q_w, k_w, q_b, k_b, v_w, v_b):
    """Fold dc conv into q/k projections -> composite 4x4 stride-2 weights."""
    dc_w = np.asarray(dc_w, np.float32)
    dc_b = np.asarray(dc_b, np.float32)
    q_w = np.asarray(q_w, np.float32)
    k_w = np.asarray(k_w, np.float32)
    q_b = np.asarray(q_b, np.float32)
    k_b = np.asarray(k_b, np.float32)
    v_w = np.asarray(v_w, np.float32)
    v_b = np.asarray(v_b, np.float32)

    C = dc_w.shape[1]
    Wq = np.zeros((48, C, 4, 4), np.float64)
    Wk = np.zeros((48, C, 4, 4), np.float64)
    for p in range(2):
        for qq in range(2):
            qw_pq = q_w[:, :, p, qq].astype(np.float64)
            kw_pq = k_w[:, :, p, qq].astype(np.float64)
            for dy in range(3):
                for dx in range(3):
                    dcw_dd = dc_w[:, :, dy, dx].astype(np.float64)
                    Wq[:, :, p + dy, qq + dx] += qw_pq @ dcw_dd
                    Wk[:, :, p + dy, qq + dx] += kw_pq @ dcw_dd
    bq_eff = q_b + q_w.sum(axis=(2, 3)) @ dc_b
    bk_eff = k_b + k_w.sum(axis=(2, 3)) @ dc_b
    # lhsT row index = (A*4+B)*C + c', columns: k 0:48 | q 48:96
    wqk_ab = (
        np.concatenate(
            [
                Wk.transpose(2, 3, 1, 0).reshape(16 * C, 48),
                np.zeros((16 * C, 16), np.float64),
                Wq.transpose(2, 3, 1, 0).reshape(16 * C, 48),
            ],
            axis=1,
        )
        .astype(np.float32)
        .reshape(32, 128, 112)  # chunk_old = (A*4+B)*2 + h; cols 48:64 zero
    )
    # permute chunks into device consumption order (h, pl, u, v)
    perm = []
    for h in range(2):
        for pl in range(4):
            a, p = divmod(pl, 2)
            for u in range(2):
                for v in range(2):
                    A, Bo = 2 * u + a, 2 * v + p
                    perm.append((A * 4 + Bo) * 2 + h)
    wqk = wqk_ab[perm].transpose(1, 0, 2)  # [part 128, chunk2 32, 96]
    bqk = np.concatenate(
        [bk_eff, np.zeros(16, np.float64), bq_eff]
    ).reshape(112, 1).astype(np.float32)
    # v rhs: row = (dy*2+dx)*C + c', col = oc -- sent bf16
    wv = np.ascontiguousarray(
        v_w.transpose(2, 3, 1, 0).reshape(8, 128, 4 * C).transpose(1, 0, 2)
    ).astype(ml_dtypes.bfloat16)  # [part 128, chunk 8, oc]
    bvr = np.ascontiguousarray(np.broadcast_to(v_b, (128, 4 * C))).astype(np.float32)
    return wqk, bqk, wv, bvr


_PROGRAM = None
LAST_RESULTS = None


def _get_program():
    global _PROGRAM
    if _PROGRAM is None:
        _PROGRAM = build_program()
    return _PROGRAM


def kernel(x, dc_w, dc_b, q_w, q_b, k_w, k_b, v_w, v_b):
    _install_ntff_hook_shim()
    x = np.asarray(x, np.float32)
    B = x.shape[0]
    xp = np.pad(x, ((0, 0), (0, 0), (1, 1), (1, 1)))
    # parity planes: xq[b, c, a*2+p, r, s] = x_pad[b, c, 2r+a, 2s+p]
    xq = (
        xp.reshape(B, C_IN, 33, 2, 33, 2)
        .transpose(0, 1, 3, 5, 2, 4)
        .reshape(B, C_IN, 4, 33, 33)
    )
    wqk, bqk, wv, bvr = host_weights(dc_w, dc_b, q_w, k_w, q_b, k_b, v_w, v_b)

    nc = _get_program()
    in_maps = []
    for c in range(NCORES):
        in_maps.append(
            {
                "xq": np.ascontiguousarray(xq[BPC * c : BPC * (c + 1)]),
                "wqk": wqk,
                "wv": wv,
                "bqk": bqk,
                "bvr": bvr,
            }
        )
    res = bass_utils.run_bass_kernel_spmd(nc, in_maps, core_ids=list(range(NCORES)))
    global LAST_RESULTS
    LAST_RESULTS = res

    out = np.empty((B, 1024, 1024), np.float32)
    for c in range(NCORES):
        out[BPC * c : BPC * (c + 1)] = (
            np.asarray(res.results[c]["o"]).astype(np.float32).transpose(0, 2, 1)
        )
    return out
